# revision 53
# baseline (speedup 1.0000x reference)
"""Trainium2 Bass kernel for nn_DualBranchDecoder.

Dual-branch band-split decoder: per-band GroupNorm -> fc1(C=128->H=512)+tanh
-> per-band fc2(H->w_k) -> sigmoid mag mask / tanh phase offset -> complex out.

Sharding: data-parallel over batch B=8 across 8 NeuronCores (one sample per
core).

v3 design notes (Activation engine is the bottleneck: tanh over 16.25M h1
elements at (N+352)/1.2 ns per instr):
 - GroupNorm folded into fc1: matmul runs on RAW fp16 features; the per-band
   1/std becomes the activation's `scale` operand and the exact per-(k,h)
   bias (b1p - mean/std * sum_c W1g) is PRELOADED into PSUM by a
   contraction-1 matmul (stationary = transposed fp16 bias row, moving =
   ones) before each fc1 chunk accumulates on top (start=False). This keeps
   tanh bias-free, legalizing 1024-wide tanh instructions: 124 instead of
   248 ACT instructions. The PE has the headroom: dep-free matmuls issue at
   216 ns (measured), so 248 extra 512-col preloads cost ~54us against the
   ~36us saved on the bottleneck ACT engine.
 - Stats (bn_stats/bn_aggr + rsqrt chain) batched per quad across both
   branches; inv and -mean*inv broadcast in one PE ones-matmul.
 - mag sigmoid computed as tanh(0.5x+0.5b) (finished in final assembly), so
   the ACT table stays on {tanh, sin}: no mid-kernel table swaps.
 - cos path: +pi/2 folded into the range-reduction adds, so sin+cos share one
   bias-free 1024-wide Sin activation per f-chunk.
 - j=0 final assembly runs right after quad 3 (bands 0-15 cover f<128).
"""
import sys
sys.path.insert(0, '/opt/trn_rl_repo')

import numpy as np

import concourse.bacc as bacc
import concourse.tile as tile
import concourse.mybir as mybir
from concourse.bass_utils import run_bass_kernel_spmd

F32 = mybir.dt.float32
FP16 = mybir.dt.float16
AF = mybir.ActivationFunctionType
ALU = mybir.AluOpType
I32 = mybir.dt.int32

# problem constants (hardcoded per contract)
B, C, T = 8, 128, 512
BANDS = [2] + [3] * 10 + [8] * 12 + [16] * 7 + [17]
K = len(BANDS)                      # 31
F = sum(BANDS)                      # 257
H = 4 * C                           # 512
NHC = H // 128                      # 4 h-chunks
EPS = 1e-5

OFFS = np.concatenate([[0], np.cumsum(BANDS)]).astype(int)   # band start freqs
WPADS = [w + (w & 1) for w in BANDS]                         # even-M pad
WOFFS = np.concatenate([[0], np.cumsum(WPADS)]).astype(int)
WPTOT = int(WOFFS[-1])

QUADS = [(4 * i, 4) for i in range(7)] + [(28, 3)]
MAGIC = float(1.5 * 2 ** 23)
INV2PI = float(1.0 / (2 * np.pi))
N2PI = float(-2 * np.pi)
PI = float(np.pi)
HALFPI = float(np.pi / 2)

_cache = {}


def _prep_branch(gamma, beta, W1, b1, W2, b2):
    """Host-side constant prep for one branch."""
    # W1gT[c, k*H + h] = W1[k,h,c] * gamma[k,c]
    W1g = W1 * gamma[:, None, :]                      # [K, H, C]
    W1gT = np.ascontiguousarray(W1g.transpose(2, 0, 1).reshape(C, K * H))
    W1gT = W1gT.astype(np.float16)
    # b1p[k,h] = b1[k,h] + sum_c W1[k,h,c]*beta[k,c]
    b1p = b1 + np.einsum('khc,kc->kh', W1, beta)      # [K, H]
    # w1sum[k,h] = sum_c W1g[k,h,c]  (fp16-rounded W1g)
    w1s = W1gT.astype(np.float32).reshape(C, K, H).sum(axis=0)   # [K, H]
    # bw row: per quad, [b1p bands k0..k0+nb | w1sum same], fp16
    bw = np.zeros((1, 2 * K * H), np.float16)
    for q, (k0, nb) in enumerate(QUADS):
        o = 2 * H * k0
        bw[0, o:o + nb * H] = b1p[k0:k0 + nb].reshape(-1).astype(np.float16)
        bw[0, o + nb * H:o + 2 * nb * H] = \
            w1s[k0:k0 + nb].reshape(-1).astype(np.float16)
    b2p = b2.astype(np.float32)
    # W2Tp[p, hc*WPTOT + woff_k + j] = W2[off_k + j, hc*128 + p], zero-pad odd
    W2Tp = np.zeros((128, NHC * WPTOT), np.float32)
    for k in range(K):
        w, off, woff = BANDS[k], OFFS[k], WOFFS[k]
        for hc in range(NHC):
            W2Tp[:, hc * WPTOT + woff: hc * WPTOT + woff + w] = \
                W2[off:off + w, hc * 128:(hc + 1) * 128].T
    W2Tp = W2Tp.astype(np.float16)
    # b2g[32*r + p, q] = b2p[off_{k0+r} + p] (p < w) for quad q
    # (the mag branch later halves this: sigmoid(x+b) == 0.5(1+tanh(.5x+.5b)))
    b2g = np.zeros((128, len(QUADS)), np.float32)
    for q, (k0, nb) in enumerate(QUADS):
        for r in range(nb):
            k = k0 + r
            b2g[32 * r:32 * r + BANDS[k], q] = b2p[OFFS[k]:OFFS[k] + BANDS[k]]
    return W1gT, bw, b2g, W2Tp


def _build():
    nc = bacc.Bacc("TRN2", target_bir_lowering=False)

    ins = {}
    for br in ("m", "p"):
        ins[f"feat_{br}"] = nc.dram_tensor(f"feat_{br}", [C, K * T], FP16,
                                           kind="ExternalInput")
        ins[f"w1gt_{br}"] = nc.dram_tensor(f"w1gt_{br}", [C, K * H], FP16,
                                           kind="ExternalInput")
        ins[f"w2tp_{br}"] = nc.dram_tensor(f"w2tp_{br}", [128, NHC * WPTOT],
                                           FP16, kind="ExternalInput")
        # per-quad blocks: [b1p bands k0..k0+nb (nb*H) | w1sum same (nb*H)]
        ins[f"bw_{br}"] = nc.dram_tensor(f"bw_{br}", [1, 2 * K * H], FP16,
                                         kind="ExternalInput")
        ins[f"b2c_{br}"] = nc.dram_tensor(f"b2c_{br}", [128, len(QUADS)], F32,
                                          kind="ExternalInput")
        ins[f"noisy_{br}"] = nc.dram_tensor(f"noisy_{br}", [F, T], F32,
                                            kind="ExternalInput")
    ones_col_d = nc.dram_tensor("ones_col", [128, 1], F32, kind="ExternalInput")
    ones_row_d = nc.dram_tensor("ones_row", [1, 128], F32, kind="ExternalInput")
    ones512_d = nc.dram_tensor("ones512", [128, 512], FP16,
                               kind="ExternalInput")
    out_d = nc.dram_tensor("out", [F, 2 * T], F32, kind="ExternalOutput")
    dbg_d = nc.dram_tensor("dbg", [128, 8 * T], F32, kind="ExternalOutput")

    with tile.TileContext(nc) as tc:
        with (
            tc.tile_pool(name="featk", bufs=3) as featk_pool,
            tc.tile_pool(name="w1t", bufs=3) as w1t_pool,
            tc.tile_pool(name="h1sb", bufs=4) as h1sb_pool,
            tc.tile_pool(name="band", bufs=4) as band_pool,
            tc.tile_pool(name="const", bufs=1) as const_pool,
            tc.tile_pool(name="statsb", bufs=2) as stats_pool,
            tc.tile_pool(name="fin", bufs=1) as fin_pool,
            tc.tile_pool(name="mainps", bufs=1, space="PSUM") as main_ps,
        ):
            # ---- critical path first: quad-0 w1(m) for warmup, then feat ----
            k0_0, nb_0 = QUADS[0]
            wq0 = w1t_pool.tile([128, nb_0 * H], FP16, tag="w1q", name="w1q_m_0")
            nc.sync.dma_start(wq0[:], ins["w1gt_m"][:, k0_0 * H:(k0_0 + nb_0) * H])
            fq0 = featk_pool.tile([128, nb_0 * T], FP16, tag="featq",
                                  name="featq_m_0")
            nc.sync.dma_start(fq0[:], ins["feat_m"][:, k0_0 * T:(k0_0 + nb_0) * T])
            fq0p = featk_pool.tile([128, nb_0 * T], FP16, tag="featq",
                                   name="featq_p_0")
            nc.sync.dma_start(fq0p[:], ins["feat_p"][:, k0_0 * T:(k0_0 + nb_0) * T])

            # ---- constants ----
            ones_col = const_pool.tile([128, 1], F32)
            nc.sync.dma_start(ones_col[:], ones_col_d[:])
            ones_row = const_pool.tile([1, 128], F32)
            nc.sync.dma_start(ones_row[:], ones_row_d[:])
            ones512 = const_pool.tile([128, 512], FP16)
            nc.sync.dma_start(ones512[:], ones512_d[:])

            cb = {}
            for br in ("m", "p"):
                w2tp = const_pool.tile([128, NHC * WPTOT], FP16,
                                       tag=f"w2tp_{br}", name=f"w2tp_{br}")
                nc.gpsimd.dma_start(w2tp[:], ins[f"w2tp_{br}"][:])
                b2c = const_pool.tile([128, len(QUADS)], F32, tag=f"b2c_{br}",
                                      name=f"b2c_{br}")
                nc.gpsimd.dma_start(b2c[:], ins[f"b2c_{br}"][:])
                cb[br] = (w2tp, b2c)

            # ---- PE warm-up: ~3.5us of matmul to trip HAM un-throttle ----
            for wi in range(16):
                wps = main_ps.tile([128, 1024], F32, tag="h1ps2", bufs=3,
                                   name=f"warm_{wi}")
                nc.tensor.matmul(wps[:, 0:512], wq0[:, 0:128], wq0[:, 0:512],
                                 start=True, stop=True)

            masks = {}
            for br in ("m", "p"):
                masks[br] = const_pool.tile([128, 2 * T], F32,
                                            tag=f"mask_{br}", name=f"mask_{br}")
                masks[br + "2"] = const_pool.tile([1, T], F32,
                                                  tag=f"mask2_{br}",
                                                  name=f"mask2_{br}")

            def final_chunk(j):
                """Complex assembly for f-chunk j (0,1: 128 rows; 2: 1 row)."""
                rows = 128 if j < 2 else 1
                if j < 2:
                    mask_ap = masks["m"][:, j * T:(j + 1) * T]
                    poff_ap = masks["p"][:, j * T:(j + 1) * T]
                else:
                    mask_ap = masks["m2"][0:1, :]
                    poff_ap = masks["p2"][0:1, :]
                nmag = fin_pool.tile([rows, T], F32, tag="nmag",
                                     name=f"nmag_{j}")
                nc.gpsimd.dma_start(nmag[:], ins["noisy_m"][j * 128:j * 128 + rows, :])
                nph = fin_pool.tile([rows, T], F32, tag="nph", name=f"nph_{j}")
                nc.gpsimd.dma_start(nph[:], ins["noisy_p"][j * 128:j * 128 + rows, :])

                ang = fin_pool.tile([rows, T], F32, tag="ang", name=f"ang_{j}")
                nc.vector.scalar_tensor_tensor(ang[:], poff_ap, PI, nph[:],
                                               op0=ALU.mult, op1=ALU.add)
                # mag mask stored as t=tanh(0.5x+0.5 b2): sigmoid = 0.5(1+t)
                nmag_h = fin_pool.tile([rows, T], F32, tag="nmag_h",
                                       name=f"nmag_h_{j}")
                nc.vector.tensor_scalar_mul(nmag_h[:], nmag[:], 0.5)
                enh = fin_pool.tile([rows, T], F32, tag="enh", name=f"enh_{j}")
                nc.vector.scalar_tensor_tensor(enh[:], mask_ap, 1.0, nmag_h[:],
                                               op0=ALU.add, op1=ALU.mult)
                # sincos input tile: cols 0:T = sin arg, T:2T = cos arg
                sc_in = fin_pool.tile([rows, 2 * T], F32, tag="sc_in",
                                      name=f"sc_in_{j}")
                sc_out = fin_pool.tile([rows, 2 * T], F32, tag="sc_out",
                                       name=f"sc_out_{j}")
                # sin: n = round(ang/2pi) via magic; ws = ang - 2pi*n
                t2 = fin_pool.tile([rows, T], F32, tag="t2", name=f"t2_{j}")
                nc.vector.tensor_scalar(t2[:], ang[:], INV2PI, MAGIC,
                                        op0=ALU.mult, op1=ALU.add)
                m2pin = fin_pool.tile([rows, T], F32, tag="m2pin",
                                      name=f"m2pin_{j}")
                nc.vector.tensor_scalar(m2pin[:], t2[:], MAGIC, N2PI,
                                        op0=ALU.subtract, op1=ALU.mult)
                nc.vector.tensor_add(sc_in[:, 0:T], ang[:], m2pin[:])
                # cos: n' = round(ang/2pi + 0.25); arg = ang + pi/2 - 2pi*n'
                t2c = fin_pool.tile([rows, T], F32, tag="t2", name=f"t2c_{j}")
                nc.vector.tensor_scalar(t2c[:], ang[:], INV2PI, 0.25,
                                        op0=ALU.mult, op1=ALU.add)
                nc.vector.tensor_scalar_add(t2c[:], t2c[:], MAGIC)
                m2pinc = fin_pool.tile([rows, T], F32, tag="m2pin",
                                       name=f"m2pinc_{j}")
                nc.vector.tensor_scalar(m2pinc[:], t2c[:], MAGIC, N2PI,
                                        op0=ALU.subtract, op1=ALU.mult)
                nc.vector.scalar_tensor_tensor(sc_in[:, T:2 * T], ang[:],
                                               HALFPI, m2pinc[:],
                                               op0=ALU.add, op1=ALU.add)
                nc.scalar.activation(sc_out[:], sc_in[:], AF.Sin)

                ot = fin_pool.tile([rows, 2 * T], F32, tag="ot", name=f"ot_{j}")
                ot2 = ot[:].rearrange("p (t two) -> p t two", two=2)
                nc.vector.tensor_mul(ot2[:, :, 0], enh[:], sc_out[:, T:2 * T])
                nc.vector.tensor_mul(ot2[:, :, 1], enh[:], sc_out[:, 0:T])
                nc.sync.dma_start(out_d[j * 128:j * 128 + rows, :], ot[:])
                if j == 0:
                    # debug: masks + sin in/out for the first f-chunk
                    nc.sync.dma_start(dbg_d[:, 0:2 * T], masks["m"][:])
                    nc.sync.dma_start(dbg_d[:, 2 * T:4 * T], masks["p"][:])
                    nc.sync.dma_start(dbg_d[:, 4 * T:6 * T], sc_in[:])
                    nc.sync.dma_start(dbg_d[:, 6 * T:8 * T], sc_out[:])

            # ---- software-pipelined quads: stats/bias prefetched one
            # quad ahead so the PE never waits on the DVE chain ----
            def stats_A(q):
                """DMAs + per-partition stats + sums for quad q (DVE only)."""
                k0, nb = QUADS[q]
                fqs, wqs, bwq = {}, {}, {}
                for br in ("m", "p"):
                    if q == 0 and br == "m":
                        fqs[br], wqs[br] = fq0, wq0
                    else:
                        if q == 0 and br == "p":
                            fqs[br] = fq0p
                        else:
                            fqs[br] = featk_pool.tile(
                                [128, nb * T], FP16, tag="featq",
                                name=f"featq_{br}_{q}")
                            nc.sync.dma_start(
                                fqs[br][:],
                                ins[f"feat_{br}"][:, k0 * T:(k0 + nb) * T])
                        wqs[br] = w1t_pool.tile([128, nb * H], FP16, tag="w1q",
                                                name=f"w1q_{br}_{q}")
                        nc.sync.dma_start(
                            wqs[br][:],
                            ins[f"w1gt_{br}"][:, k0 * H:(k0 + nb) * H])
                    bwq[br] = stats_pool.tile([1, 2 * nb * H], FP16, tag="bw",
                                              bufs=4, name=f"bw_{br}_{q}")
                    nc.gpsimd.dma_start(
                        bwq[br][:],
                        ins[f"bw_{br}"][:, 2 * H * k0:2 * H * (k0 + nb)])
                sums = stats_pool.tile([128, 4 * nb], F32, tag="sums", bufs=2,
                                       name=f"sums_{q}")
                for bi, br in enumerate(("m", "p")):
                    st_q = stats_pool.tile([128, nb * 6], F32, tag="st_q",
                                           bufs=2, name=f"st_{br}_{q}")
                    ag_q = stats_pool.tile([128, nb * 2], F32, tag="ag_q",
                                           bufs=2, name=f"ag_{br}_{q}")
                    for r in range(nb):
                        nc.vector.bn_stats(st_q[:, r * 6:(r + 1) * 6],
                                           fqs[br][:, r * T:(r + 1) * T])
                        nc.vector.bn_aggr(ag_q[:, r * 2:(r + 1) * 2],
                                          st_q[:, r * 6:(r + 1) * 6])
                    ag3 = ag_q[:].rearrange("c (k two) -> c k two", two=2)
                    mean_ap = ag3[:, :, 0]
                    var_ap = ag3[:, :, 1]
                    o = 2 * nb * bi
                    nc.vector.tensor_copy(sums[:, o:o + nb], mean_ap)
                    tmp = stats_pool.tile([128, nb], F32, tag="tmp", bufs=2,
                                          name=f"tmp_{br}_{q}")
                    nc.vector.tensor_mul(tmp[:], mean_ap, mean_ap)
                    nc.vector.tensor_add(sums[:, o + nb:o + 2 * nb], tmp[:],
                                         var_ap)
                return dict(q=q, fqs=fqs, wqs=wqs, bwq=bwq, sums=sums)

            def stats_B(a):
                """Cross-partition reduce + rsqrt chain + broadcasts + per-band
                bias rows for quad a['q'] (one tiny PE MM pair + DVE)."""
                q = a["q"]
                k0, nb = QUADS[q]
                ps_s = main_ps.tile([1, 4 * nb], F32, tag="ps_s", bufs=1,
                                    name=f"ps_s_{q}")
                nc.tensor.matmul(ps_s[:], ones_col[:], a["sums"][:],
                                 start=True, stop=True)
                g = stats_pool.tile([1, 4 * nb], F32, tag="g", bufs=2,
                                    name=f"g_{q}")
                nc.vector.tensor_scalar_mul(g[:], ps_s[:], 1.0 / C)
                gm2 = stats_pool.tile([1, 2 * nb], F32, tag="gm2", bufs=2,
                                      name=f"gm2_{q}")
                gvar = stats_pool.tile([1, 2 * nb], F32, tag="gvar", bufs=2,
                                       name=f"gvar_{q}")
                gq = g[:].rearrange("c (b two k) -> c b two k", b=2, two=2)
                nc.vector.tensor_mul(gm2[:].rearrange("c (b k) -> c b k", b=2),
                                     gq[:, :, 0, :], gq[:, :, 0, :])
                nc.vector.tensor_sub(gvar[:].rearrange("c (b k) -> c b k", b=2),
                                     gq[:, :, 1, :], gm2[:].rearrange(
                                         "c (b k) -> c b k", b=2))
                # inv = rsqrt(gvar + EPS): quake seed + 3 Newton (pure DVE)
                vv = stats_pool.tile([1, 2 * nb], F32, tag="vv", bufs=2,
                                     name=f"vv_{q}")
                nc.vector.tensor_scalar_add(vv[:], gvar[:], EPS)
                yy = stats_pool.tile([1, 2 * nb], F32, tag="yy", bufs=2,
                                     name=f"yy_{q}")
                nc.vector.tensor_scalar(yy[:].bitcast(I32), vv[:].bitcast(I32),
                                        1, -1, op0=ALU.arith_shift_right,
                                        op1=ALU.bitwise_xor)
                nc.vector.tensor_scalar_add(yy[:].bitcast(I32),
                                            yy[:].bitcast(I32), 0x5f3759e0)
                # invnim[1, 6nb]: [inv_m|inv_p | std_m|std_p | -mean_m|-mean_p]
                # PSUM preload gets biasT = b1p*std - mean*w1sum so that
                # tanh(inv*(mm + biasT)) == tanh(inv*mm + bias_true).
                invnim = stats_pool.tile([1, 6 * nb], F32, tag="invnim",
                                         bufs=3, name=f"invnim_{q}")
                tnr = stats_pool.tile([1, 2 * nb], F32, tag="tnr", bufs=2,
                                      name=f"tnr_{q}")
                for it in range(3):
                    nc.vector.tensor_mul(tnr[:], yy[:], yy[:])
                    nc.vector.tensor_mul(tnr[:], tnr[:], vv[:])
                    nc.vector.tensor_scalar(tnr[:], tnr[:], -0.5, 1.5,
                                            op0=ALU.mult, op1=ALU.add)
                    dst = yy[:] if it < 2 else invnim[:, 0:2 * nb]
                    nc.vector.tensor_mul(dst, yy[:], tnr[:])
                # std = (var+eps) * rsqrt(var+eps)
                nc.vector.tensor_mul(invnim[:, 2 * nb:4 * nb], vv[:],
                                     invnim[:, 0:2 * nb])
                nc.vector.tensor_scalar_mul(
                    invnim[:, 4 * nb:6 * nb].rearrange("c (b k) -> c b k", b=2),
                    gq[:, :, 0, :], -1.0)
                # broadcast inv to all partitions: bbq[128, 2nb]
                ps_b = main_ps.tile([128, 2 * nb], F32, tag="ps_s", bufs=1,
                                    name=f"ps_b_{q}")
                nc.tensor.matmul(ps_b[:], ones_row[:], invnim[:, 0:2 * nb],
                                 start=True, stop=True)
                bbq = stats_pool.tile([128, 2 * nb], F32, tag="bbq", bufs=3,
                                      name=f"bbq_{q}")
                nc.vector.tensor_copy(bbq[:], ps_b[:])
                # per-band fp16 bias rows (row 0 of a zeroed [128, .] tile:
                # full-height stationary keeps the preload MM HAM-visible)
                biasqTs = {}
                for bi, br in enumerate(("m", "p")):
                    bw = a["bwq"][br]
                    biasqT = stats_pool.tile([128, nb * H], FP16, tag="biasqT",
                                             bufs=4, name=f"biasqT_{br}_{q}")
                    if 2 * q + bi < 4:
                        nc.vector.memset(biasqT[:], 0.0)
                    tmpb = stats_pool.tile([1, H], FP16, tag="tmpb", bufs=4,
                                           name=f"tmpb_{br}_{q}")
                    for r in range(nb):
                        # biasT = b1p*std + (-mean)*w1sum
                        nc.vector.tensor_scalar_mul(
                            tmpb[:], bw[:, r * H:(r + 1) * H],
                            invnim[0:1, 2 * nb + bi * nb + r:
                                   2 * nb + bi * nb + r + 1])
                        nc.vector.scalar_tensor_tensor(
                            biasqT[0:1, r * H:(r + 1) * H],
                            bw[:, nb * H + r * H:nb * H + (r + 1) * H],
                            invnim[0:1, 4 * nb + bi * nb + r:
                                   4 * nb + bi * nb + r + 1],
                            tmpb[:],
                            op0=ALU.mult, op1=ALU.add)
                    biasqTs[br] = biasqT
                return dict(**a, bbq=bbq, biasqTs=biasqTs)

            def bands(st, bi, br):
                """fc1(+bias preload)+tanh+fc2+mask for one quad-branch."""
                q = st["q"]
                k0, nb = QUADS[q]
                w2tp, b2c = cb[br]
                fq, wq = st["fqs"][br], st["wqs"][br]
                bbq, biasqT = st["bbq"], st["biasqTs"][br]
                fc2g = main_ps.tile([128, T], F32, tag="fc2ps", bufs=1,
                                    name=f"fc2g_{br}_{q}")
                for r in range(nb):
                    k = k0 + r
                    h1sb = h1sb_pool.tile([128, NHC * T], FP16, bufs=6,
                                          tag="h1sb", name=f"h1sb_{br}_{k}")
                    for half in range(2):
                        hps = main_ps.tile([128, 1024], F32, tag="h1ps2",
                                           bufs=3, name=f"h1ps_{br}_{k}_{half}")
                        for hh in range(2):
                            hc = 2 * half + hh
                            # bias preload, then fc1 accumulates on top
                            nc.tensor.matmul(
                                hps[:, hh * T:(hh + 1) * T],
                                biasqT[:, (r * NHC + hc) * 128:
                                       (r * NHC + hc + 1) * 128],
                                ones512[:], start=True, stop=False)
                            nc.tensor.matmul(
                                hps[:, hh * T:(hh + 1) * T],
                                wq[:, (r * NHC + hc) * 128:
                                      (r * NHC + hc + 1) * 128],
                                fq[:, r * T:(r + 1) * T],
                                start=False, stop=True)
                        nc.scalar.activation(
                            h1sb[:, half * 1024:(half + 1) * 1024],
                            hps[:], AF.Tanh,
                            scale=bbq[:, bi * nb + r:bi * nb + r + 1])
                    wp, woff = WPADS[k], int(WOFFS[k])
                    for hc in range(NHC):
                        nc.tensor.matmul(
                            fc2g[32 * r:32 * r + wp, :],
                            w2tp[:, hc * WPTOT + woff: hc * WPTOT + woff + wp],
                            h1sb[:, hc * T:(hc + 1) * T],
                            start=(hc == 0), stop=(hc == NHC - 1),
                            tile_position=(0, 32 * r))
                # m branch: sigmoid(x+b2) computed as tanh(0.5x+0.5b2)
                # (b2c_m pre-halved on host); completed in final_chunk.
                # Keeps the ACT table on {tanh, sin} -> zero table swaps.
                grp_t = band_pool.tile([128, T], F32, tag="band",
                                       name=f"grp_{br}_{q}")
                nc.scalar.activation(grp_t[:], fc2g[:], AF.Tanh,
                                     bias=b2c[:, q:q + 1],
                                     scale=0.5 if br == "m" else 1.0)
                for r in range(nb):
                    k = k0 + r
                    w, off = BANDS[k], int(OFFS[k])
                    j0, r0 = off // 128, off % 128
                    if off + w <= (j0 + 1) * 128:
                        nc.sync.dma_start(
                            masks[br][r0:r0 + w, j0 * T:(j0 + 1) * T],
                            grp_t[32 * r:32 * r + w, :])
                    else:
                        n1 = (j0 + 1) * 128 - off
                        nc.sync.dma_start(
                            masks[br][r0:128, j0 * T:(j0 + 1) * T],
                            grp_t[32 * r:32 * r + n1, :])
                        rem = w - n1
                        if j0 + 1 < 2:
                            nc.sync.dma_start(
                                masks[br][0:rem, (j0 + 1) * T:(j0 + 2) * T],
                                grp_t[32 * r + n1:32 * r + w, :])
                        else:
                            nc.sync.dma_start(
                                masks[br + "2"][0:rem, :],
                                grp_t[32 * r + n1:32 * r + w, :])

            st = stats_B(stats_A(0))
            for q in range(len(QUADS)):
                a_next = stats_A(q + 1) if q + 1 < len(QUADS) else None
                bands(st, 0, "m")
                st_next = stats_B(a_next) if a_next is not None else None
                bands(st, 1, "p")
                st = st_next

            # all finals at kernel end: mid-kernel consumption of the
            # DMA-scattered masks races the scatters (the framework's
            # single-sem wait cannot cover many DMA writers across queues).
            final_chunk(0)
            final_chunk(1)
            final_chunk(2)

    nc.compile()
    return nc


def kernel(mag_features, phase_features, noisy_mag, noisy_phase,
           mag_gamma, mag_beta, mag_W1, mag_b1, mag_W2, mag_b2,
           ph_gamma, ph_beta, ph_W1, ph_b1, ph_W2, ph_b2):
    if "nc" not in _cache:
        _cache["nc"] = _build()
    nc = _cache["nc"]

    mW1gT, mbw, mb2c, mW2Tp = _prep_branch(
        np.asarray(mag_gamma), np.asarray(mag_beta), np.asarray(mag_W1),
        np.asarray(mag_b1), np.asarray(mag_W2), np.asarray(mag_b2))
    mb2c = mb2c * 0.5          # mag sigmoid -> tanh(0.5x + 0.5 b2) trick
    pW1gT, pbw, pb2c, pW2Tp = _prep_branch(
        np.asarray(ph_gamma), np.asarray(ph_beta), np.asarray(ph_W1),
        np.asarray(ph_b1), np.asarray(ph_W2), np.asarray(ph_b2))

    shared = dict(
        w1gt_m=mW1gT, w2tp_m=mW2Tp, b2c_m=mb2c, bw_m=mbw,
        w1gt_p=pW1gT, w2tp_p=pW2Tp, b2c_p=pb2c, bw_p=pbw,
        ones_col=np.ones((128, 1), np.float32),
        ones_row=np.ones((1, 128), np.float32),
        ones512=np.ones((128, 512), np.float16),
    )
    mag_features = np.asarray(mag_features)
    phase_features = np.asarray(phase_features)
    noisy_mag = np.asarray(noisy_mag)
    noisy_phase = np.asarray(noisy_phase)

    in_maps = []
    for b in range(B):
        m = dict(shared)
        # [C, T, K] -> [C, K, T] k-major, contiguous per-band slices, fp16
        m["feat_m"] = np.ascontiguousarray(
            mag_features[b].transpose(0, 2, 1)).reshape(C, K * T).astype(
                np.float16)
        m["feat_p"] = np.ascontiguousarray(
            phase_features[b].transpose(0, 2, 1)).reshape(C, K * T).astype(
                np.float16)
        m["noisy_m"] = np.ascontiguousarray(noisy_mag[b])
        m["noisy_p"] = np.ascontiguousarray(noisy_phase[b])
        in_maps.append(m)

    import os
    trace = bool(os.environ.get("BASS_PROFILE"))
    res = run_bass_kernel_spmd(nc, in_maps, list(range(B)), trace=trace)
    _cache["last_result"] = res
    out = np.stack([res.results[b]["out"].view(np.complex64) for b in range(B)])
    _cache["dbg"] = np.stack([res.results[b]["dbg"] for b in range(B)])
    return out


# revision 59
# speedup vs baseline: 1.0031x; 1.0031x over previous
"""Trainium2 Bass kernel for nn_DualBranchDecoder.

Dual-branch band-split decoder: per-band GroupNorm -> fc1(C=128->H=512)+tanh
-> per-band fc2(H->w_k) -> sigmoid mag mask / tanh phase offset -> complex out.

Sharding: data-parallel over batch B=8 across 8 NeuronCores (one sample per
core).

v3 design notes (Activation engine is the bottleneck: tanh over 16.25M h1
elements at (N+352)/1.2 ns per instr):
 - GroupNorm folded into fc1: matmul runs on RAW fp16 features; the per-band
   1/std becomes the activation's `scale` operand and the exact per-(k,h)
   bias (b1p - mean/std * sum_c W1g) is PRELOADED into PSUM by a
   contraction-1 matmul (stationary = transposed fp16 bias row, moving =
   ones) before each fc1 chunk accumulates on top (start=False). This keeps
   tanh bias-free, legalizing 1024-wide tanh instructions: 124 instead of
   248 ACT instructions. The PE has the headroom: dep-free matmuls issue at
   216 ns (measured), so 248 extra 512-col preloads cost ~54us against the
   ~36us saved on the bottleneck ACT engine.
 - Stats (bn_stats/bn_aggr + rsqrt chain) batched per quad across both
   branches; inv and -mean*inv broadcast in one PE ones-matmul.
 - mag sigmoid computed as tanh(0.5x+0.5b) (finished in final assembly), so
   the ACT table stays on {tanh, sin}: no mid-kernel table swaps.
 - cos path: +pi/2 folded into the range-reduction adds, so sin+cos share one
   bias-free 1024-wide Sin activation per f-chunk.
 - j=0 final assembly runs right after quad 3 (bands 0-15 cover f<128).
"""
import sys
sys.path.insert(0, '/opt/trn_rl_repo')

import numpy as np

import concourse.bacc as bacc
import concourse.tile as tile
import concourse.mybir as mybir
from concourse.bass_utils import run_bass_kernel_spmd

F32 = mybir.dt.float32
FP16 = mybir.dt.float16
AF = mybir.ActivationFunctionType
ALU = mybir.AluOpType
I32 = mybir.dt.int32

# problem constants (hardcoded per contract)
B, C, T = 8, 128, 512
BANDS = [2] + [3] * 10 + [8] * 12 + [16] * 7 + [17]
K = len(BANDS)                      # 31
F = sum(BANDS)                      # 257
H = 4 * C                           # 512
NHC = H // 128                      # 4 h-chunks
EPS = 1e-5

OFFS = np.concatenate([[0], np.cumsum(BANDS)]).astype(int)   # band start freqs
WPADS = [w + (w & 1) for w in BANDS]                         # even-M pad
WOFFS = np.concatenate([[0], np.cumsum(WPADS)]).astype(int)
WPTOT = int(WOFFS[-1])

QUADS = [(4 * i, 4) for i in range(7)] + [(28, 3)]
MAGIC = float(1.5 * 2 ** 23)
INV2PI = float(1.0 / (2 * np.pi))
N2PI = float(-2 * np.pi)
PI = float(np.pi)
HALFPI = float(np.pi / 2)

_cache = {}


def _prep_branch(gamma, beta, W1, b1, W2, b2):
    """Host-side constant prep for one branch."""
    # W1gT[c, k*H + h] = W1[k,h,c] * gamma[k,c]
    W1g = W1 * gamma[:, None, :]                      # [K, H, C]
    W1gT = np.ascontiguousarray(W1g.transpose(2, 0, 1).reshape(C, K * H))
    W1gT = W1gT.astype(np.float16)
    # b1p[k,h] = b1[k,h] + sum_c W1[k,h,c]*beta[k,c]
    b1p = b1 + np.einsum('khc,kc->kh', W1, beta)      # [K, H]
    # w1sum[k,h] = sum_c W1g[k,h,c]  (fp16-rounded W1g)
    w1s = W1gT.astype(np.float32).reshape(C, K, H).sum(axis=0)   # [K, H]
    # bw row: per quad, [b1p bands k0..k0+nb | w1sum same], fp16
    bw = np.zeros((1, 2 * K * H), np.float16)
    for q, (k0, nb) in enumerate(QUADS):
        o = 2 * H * k0
        bw[0, o:o + nb * H] = b1p[k0:k0 + nb].reshape(-1).astype(np.float16)
        bw[0, o + nb * H:o + 2 * nb * H] = \
            w1s[k0:k0 + nb].reshape(-1).astype(np.float16)
    b2p = b2.astype(np.float32)
    # broadcast-layout b1p/w1sum for the AP-mode bands: [128, K*NHC] f32
    b1c = np.zeros((128, K * NHC), np.float32)
    w1c = np.zeros((128, K * NHC), np.float32)
    for k in range(K):
        for hc in range(NHC):
            b1c[:, k * NHC + hc] = b1p[k, hc * 128:(hc + 1) * 128]
            w1c[:, k * NHC + hc] = w1s[k, hc * 128:(hc + 1) * 128]
    # W2Tp[p, hc*WPTOT + woff_k + j] = W2[off_k + j, hc*128 + p], zero-pad odd
    W2Tp = np.zeros((128, NHC * WPTOT), np.float32)
    for k in range(K):
        w, off, woff = BANDS[k], OFFS[k], WOFFS[k]
        for hc in range(NHC):
            W2Tp[:, hc * WPTOT + woff: hc * WPTOT + woff + w] = \
                W2[off:off + w, hc * 128:(hc + 1) * 128].T
    W2Tp = W2Tp.astype(np.float16)
    # b2g[32*r + p, q] = b2p[off_{k0+r} + p] (p < w) for quad q
    # (the mag branch later halves this: sigmoid(x+b) == 0.5(1+tanh(.5x+.5b)))
    b2g = np.zeros((128, len(QUADS)), np.float32)
    for q, (k0, nb) in enumerate(QUADS):
        for r in range(nb):
            k = k0 + r
            b2g[32 * r:32 * r + BANDS[k], q] = b2p[OFFS[k]:OFFS[k] + BANDS[k]]
    return W1gT, bw, b1c, w1c, b2g, W2Tp


def _build():
    nc = bacc.Bacc("TRN2", target_bir_lowering=False)

    ins = {}
    for br in ("m", "p"):
        ins[f"feat_{br}"] = nc.dram_tensor(f"feat_{br}", [C, K * T], FP16,
                                           kind="ExternalInput")
        ins[f"w1gt_{br}"] = nc.dram_tensor(f"w1gt_{br}", [C, K * H], FP16,
                                           kind="ExternalInput")
        ins[f"w2tp_{br}"] = nc.dram_tensor(f"w2tp_{br}", [128, NHC * WPTOT],
                                           FP16, kind="ExternalInput")
        # per-quad blocks: [b1p bands k0..k0+nb (nb*H) | w1sum same (nb*H)]
        ins[f"bw_{br}"] = nc.dram_tensor(f"bw_{br}", [1, 2 * K * H], FP16,
                                         kind="ExternalInput")
        ins[f"b2c_{br}"] = nc.dram_tensor(f"b2c_{br}", [128, len(QUADS)], F32,
                                          kind="ExternalInput")
        ins[f"b1c_{br}"] = nc.dram_tensor(f"b1c_{br}", [128, K * NHC], F32,
                                          kind="ExternalInput")
        ins[f"w1c_{br}"] = nc.dram_tensor(f"w1c_{br}", [128, K * NHC], F32,
                                          kind="ExternalInput")
        ins[f"noisy_{br}"] = nc.dram_tensor(f"noisy_{br}", [F, T], F32,
                                            kind="ExternalInput")
    ones_col_d = nc.dram_tensor("ones_col", [128, 1], F32, kind="ExternalInput")
    ones_row_d = nc.dram_tensor("ones_row", [1, 128], F32, kind="ExternalInput")
    ones512_d = nc.dram_tensor("ones512", [128, 512], FP16,
                               kind="ExternalInput")
    out_d = nc.dram_tensor("out", [F, 2 * T], F32, kind="ExternalOutput")

    with tile.TileContext(nc) as tc:
        with (
            tc.tile_pool(name="featk", bufs=3) as featk_pool,
            tc.tile_pool(name="w1t", bufs=3) as w1t_pool,
            tc.tile_pool(name="h1sb", bufs=4) as h1sb_pool,
            tc.tile_pool(name="band", bufs=4) as band_pool,
            tc.tile_pool(name="const", bufs=1) as const_pool,
            tc.tile_pool(name="statsb", bufs=2) as stats_pool,
            tc.tile_pool(name="fin", bufs=1) as fin_pool,
            tc.tile_pool(name="mainps", bufs=1, space="PSUM") as main_ps,
        ):
            # ---- critical path first: quad-0 w1(m) for warmup, then feat ----
            k0_0, nb_0 = QUADS[0]
            wq0 = w1t_pool.tile([128, nb_0 * H], FP16, tag="w1q", name="w1q_m_0")
            nc.sync.dma_start(wq0[:], ins["w1gt_m"][:, k0_0 * H:(k0_0 + nb_0) * H])
            fq0 = featk_pool.tile([128, nb_0 * T], FP16, tag="featq",
                                  name="featq_m_0")
            nc.sync.dma_start(fq0[:], ins["feat_m"][:, k0_0 * T:(k0_0 + nb_0) * T])
            fq0p = featk_pool.tile([128, nb_0 * T], FP16, tag="featq",
                                   name="featq_p_0")
            nc.sync.dma_start(fq0p[:], ins["feat_p"][:, k0_0 * T:(k0_0 + nb_0) * T])

            # ---- constants ----
            ones_col = const_pool.tile([128, 1], F32)
            nc.sync.dma_start(ones_col[:], ones_col_d[:])
            ones_row = const_pool.tile([1, 128], F32)
            nc.sync.dma_start(ones_row[:], ones_row_d[:])
            ones512 = const_pool.tile([128, 512], FP16)
            nc.sync.dma_start(ones512[:], ones512_d[:])

            cb, cb2 = {}, {}
            for br in ("m", "p"):
                w2tp = const_pool.tile([128, NHC * WPTOT], FP16,
                                       tag=f"w2tp_{br}", name=f"w2tp_{br}")
                nc.gpsimd.dma_start(w2tp[:], ins[f"w2tp_{br}"][:])
                b2c = const_pool.tile([128, len(QUADS)], F32, tag=f"b2c_{br}",
                                      name=f"b2c_{br}")
                nc.gpsimd.dma_start(b2c[:], ins[f"b2c_{br}"][:])
                b1c = const_pool.tile([128, K * NHC], F32, tag=f"b1c_{br}",
                                      name=f"b1c_{br}")
                nc.gpsimd.dma_start(b1c[:], ins[f"b1c_{br}"][:])
                w1c = const_pool.tile([128, K * NHC], F32, tag=f"w1c_{br}",
                                      name=f"w1c_{br}")
                nc.gpsimd.dma_start(w1c[:], ins[f"w1c_{br}"][:])
                cb[br] = (w2tp, b2c)
                cb2[br] = (b1c, w1c)

            # ---- PE warm-up: keep the PE busy until the first real fc1
            # matmuls (stats chain latency ~12us) so HAM stays at K=8 ----
            for wi in range(48):
                wps = main_ps.tile([128, 1024], F32, tag="h1ps2", bufs=3,
                                   name=f"warm_{wi}")
                nc.tensor.matmul(wps[:, 0:512], wq0[:, 0:128], wq0[:, 0:512],
                                 start=True, stop=True)

            masks = {}
            for br in ("m", "p"):
                masks[br] = const_pool.tile([128, 2 * T], F32,
                                            tag=f"mask_{br}", name=f"mask_{br}")
                masks[br + "2"] = const_pool.tile([1, T], F32,
                                                  tag=f"mask2_{br}",
                                                  name=f"mask2_{br}")

            # noisy inputs prefetched well before the finals need them
            noisy_t = {}
            for j in range(3):
                rows = 128 if j < 2 else 1
                for nm, src in (("nmag", "noisy_m"), ("nph", "noisy_p")):
                    tl = fin_pool.tile([rows, T], F32, tag=f"{nm}_{j}",
                                       name=f"{nm}_{j}")
                    nc.gpsimd.dma_start(
                        tl[:], ins[src][j * 128:j * 128 + rows, :])
                    noisy_t[(nm, j)] = tl

            def final_chunk(j):
                """Complex assembly for f-chunk j (0,1: 128 rows; 2: 1 row).
                noisy_m arrives pre-halved from the host: sigmoid mask
                = 0.5(1+t) so enh = (t+1)*(0.5*noisy)."""
                ve = nc.vector
                rows = 128 if j < 2 else 1
                if j < 2:
                    mask_ap = masks["m"][:, j * T:(j + 1) * T]
                    poff_ap = masks["p"][:, j * T:(j + 1) * T]
                else:
                    mask_ap = masks["m2"][0:1, :]
                    poff_ap = masks["p2"][0:1, :]
                nmag = noisy_t[("nmag", j)]
                nph = noisy_t[("nph", j)]

                ang = fin_pool.tile([rows, T], F32, tag=f"ang_{j}",
                                    name=f"ang_{j}")
                ve.scalar_tensor_tensor(ang[:], poff_ap, PI, nph[:],
                                        op0=ALU.mult, op1=ALU.add)
                enh = fin_pool.tile([rows, T], F32, tag=f"enh_{j}",
                                    name=f"enh_{j}")
                ve.scalar_tensor_tensor(enh[:], mask_ap, 1.0, nmag[:],
                                        op0=ALU.add, op1=ALU.mult)
                # sincos input tile: cols 0:T = sin arg, T:2T = cos arg
                sc_in = fin_pool.tile([rows, 2 * T], F32, tag=f"sc_in_{j}",
                                      name=f"sc_in_{j}")
                sc_out = fin_pool.tile([rows, 2 * T], F32, tag=f"sc_out_{j}",
                                       name=f"sc_out_{j}")
                # sin: n = round(ang/2pi) via magic; ws = ang - 2pi*n
                t2 = fin_pool.tile([rows, T], F32, tag=f"t2_{j}",
                                   name=f"t2_{j}")
                ve.tensor_scalar(t2[:], ang[:], INV2PI, MAGIC,
                                 op0=ALU.mult, op1=ALU.add)
                m2pin = fin_pool.tile([rows, T], F32, tag=f"m2pin_{j}",
                                      name=f"m2pin_{j}")
                ve.tensor_scalar(m2pin[:], t2[:], MAGIC, N2PI,
                                 op0=ALU.subtract, op1=ALU.mult)
                ve.tensor_add(sc_in[:, 0:T], ang[:], m2pin[:])
                # cos: sin(angc - 2pi*round(angc/2pi)) with angc = ang + pi/2
                angc = fin_pool.tile([rows, T], F32, tag=f"angc_{j}",
                                     name=f"angc_{j}")
                ve.tensor_scalar_add(angc[:], ang[:], HALFPI)
                t2c = fin_pool.tile([rows, T], F32, tag=f"t2_{j}",
                                    name=f"t2c_{j}")
                ve.tensor_scalar(t2c[:], angc[:], INV2PI, MAGIC,
                                 op0=ALU.mult, op1=ALU.add)
                m2pinc = fin_pool.tile([rows, T], F32, tag=f"m2pin_{j}",
                                       name=f"m2pinc_{j}")
                ve.tensor_scalar(m2pinc[:], t2c[:], MAGIC, N2PI,
                                 op0=ALU.subtract, op1=ALU.mult)
                ve.tensor_add(sc_in[:, T:2 * T], angc[:], m2pinc[:])
                nc.scalar.activation(sc_out[:], sc_in[:], AF.Sin)

                ot = fin_pool.tile([rows, 2 * T], F32, tag=f"ot_{j}",
                                   name=f"ot_{j}")
                ot2 = ot[:].rearrange("p (t two) -> p t two", two=2)
                ve.tensor_mul(ot2[:, :, 0], enh[:], sc_out[:, T:2 * T])
                ve.tensor_mul(ot2[:, :, 1], enh[:], sc_out[:, 0:T])
                nc.sync.dma_start(out_d[j * 128:j * 128 + rows, :], ot[:])

            # ---- software-pipelined quads: stats/bias prefetched one
            # quad ahead so the PE never waits on the DVE chain ----
            def stats_A(q):
                """DMAs + per-partition stats + sums for quad q (DVE only)."""
                k0, nb = QUADS[q]
                fqs, wqs, bwq = {}, {}, {}
                for br in ("m", "p"):
                    if q == 0 and br == "m":
                        fqs[br], wqs[br] = fq0, wq0
                    else:
                        if q == 0 and br == "p":
                            fqs[br] = fq0p
                        else:
                            fqs[br] = featk_pool.tile(
                                [128, nb * T], FP16, tag="featq",
                                name=f"featq_{br}_{q}")
                            nc.sync.dma_start(
                                fqs[br][:],
                                ins[f"feat_{br}"][:, k0 * T:(k0 + nb) * T])
                        wqs[br] = w1t_pool.tile([128, nb * H], FP16, tag="w1q",
                                                name=f"w1q_{br}_{q}")
                        nc.sync.dma_start(
                            wqs[br][:],
                            ins[f"w1gt_{br}"][:, k0 * H:(k0 + nb) * H])
                    bwq[br] = stats_pool.tile([1, 2 * nb * H], FP16, tag="bw",
                                              bufs=4, name=f"bw_{br}_{q}")
                    nc.gpsimd.dma_start(
                        bwq[br][:],
                        ins[f"bw_{br}"][:, 2 * H * k0:2 * H * (k0 + nb)])
                sums = stats_pool.tile([128, 4 * nb], F32, tag="sums", bufs=2,
                                       name=f"sums_{q}")
                for bi, br in enumerate(("m", "p")):
                    st_q = stats_pool.tile([128, nb * 6], F32, tag="st_q",
                                           bufs=2, name=f"st_{br}_{q}")
                    ag_q = stats_pool.tile([128, nb * 2], F32, tag="ag_q",
                                           bufs=2, name=f"ag_{br}_{q}")
                    for r in range(nb):
                        nc.vector.bn_stats(st_q[:, r * 6:(r + 1) * 6],
                                           fqs[br][:, r * T:(r + 1) * T])
                        nc.vector.bn_aggr(ag_q[:, r * 2:(r + 1) * 2],
                                          st_q[:, r * 6:(r + 1) * 6])
                    ag3 = ag_q[:].rearrange("c (k two) -> c k two", two=2)
                    mean_ap = ag3[:, :, 0]
                    var_ap = ag3[:, :, 1]
                    o = 2 * nb * bi
                    nc.vector.tensor_copy(sums[:, o:o + nb], mean_ap)
                    tmp = stats_pool.tile([128, nb], F32, tag="tmp", bufs=2,
                                          name=f"tmp_{br}_{q}")
                    nc.vector.tensor_mul(tmp[:], mean_ap, mean_ap)
                    nc.vector.tensor_add(sums[:, o + nb:o + 2 * nb], tmp[:],
                                         var_ap)
                return dict(q=q, fqs=fqs, wqs=wqs, bwq=bwq, sums=sums)

            def stats_B(a):
                """Cross-partition reduce + rsqrt chain + broadcasts + per-band
                bias rows for quad a['q'] (one tiny PE MM pair + DVE)."""
                q = a["q"]
                k0, nb = QUADS[q]
                ps_s = main_ps.tile([1, 4 * nb], F32, tag="ps_s", bufs=1,
                                    name=f"ps_s_{q}")
                nc.tensor.matmul(ps_s[:], ones_col[:], a["sums"][:],
                                 start=True, stop=True)
                g = stats_pool.tile([1, 4 * nb], F32, tag="g", bufs=2,
                                    name=f"g_{q}")
                nc.vector.tensor_scalar_mul(g[:], ps_s[:], 1.0 / C)
                gm2 = stats_pool.tile([1, 2 * nb], F32, tag="gm2", bufs=2,
                                      name=f"gm2_{q}")
                gvar = stats_pool.tile([1, 2 * nb], F32, tag="gvar", bufs=2,
                                       name=f"gvar_{q}")
                gq = g[:].rearrange("c (b two k) -> c b two k", b=2, two=2)
                nc.vector.tensor_mul(gm2[:].rearrange("c (b k) -> c b k", b=2),
                                     gq[:, :, 0, :], gq[:, :, 0, :])
                nc.vector.tensor_sub(gvar[:].rearrange("c (b k) -> c b k", b=2),
                                     gq[:, :, 1, :], gm2[:].rearrange(
                                         "c (b k) -> c b k", b=2))
                # inv = rsqrt(gvar + EPS): quake seed + 3 Newton (pure DVE)
                vv = stats_pool.tile([1, 2 * nb], F32, tag="vv", bufs=2,
                                     name=f"vv_{q}")
                nc.vector.tensor_scalar_add(vv[:], gvar[:], EPS)
                yy = stats_pool.tile([1, 2 * nb], F32, tag="yy", bufs=2,
                                     name=f"yy_{q}")
                nc.vector.tensor_scalar(yy[:].bitcast(I32), vv[:].bitcast(I32),
                                        1, -1, op0=ALU.arith_shift_right,
                                        op1=ALU.bitwise_xor)
                nc.vector.tensor_scalar_add(yy[:].bitcast(I32),
                                            yy[:].bitcast(I32), 0x5f3759e0)
                # invnim[1, 8nb]: [inv | nim=-mean*inv | std | -mean] per
                # branch pair. PSUM preload gets biasT = b1p*std - mean*w1sum
                # so tanh(inv*(mm + biasT)) == tanh(inv*mm + bias_true); the
                # AP-mode bands use bias = b1p + nim*w1sum directly.
                invnim = stats_pool.tile([1, 8 * nb], F32, tag="invnim",
                                         bufs=3, name=f"invnim_{q}")
                tnr = stats_pool.tile([1, 2 * nb], F32, tag="tnr", bufs=2,
                                      name=f"tnr_{q}")
                for it in range(3):
                    nc.vector.tensor_mul(tnr[:], yy[:], yy[:])
                    nc.vector.tensor_mul(tnr[:], tnr[:], vv[:])
                    nc.vector.tensor_scalar(tnr[:], tnr[:], -0.5, 1.5,
                                            op0=ALU.mult, op1=ALU.add)
                    dst = yy[:] if it < 2 else invnim[:, 0:2 * nb]
                    nc.vector.tensor_mul(dst, yy[:], tnr[:])
                # std = (var+eps) * rsqrt(var+eps)
                nc.vector.tensor_mul(invnim[:, 4 * nb:6 * nb], vv[:],
                                     invnim[:, 0:2 * nb])
                nc.vector.tensor_scalar_mul(
                    invnim[:, 6 * nb:8 * nb].rearrange("c (b k) -> c b k", b=2),
                    gq[:, :, 0, :], -1.0)
                nc.vector.tensor_mul(invnim[:, 2 * nb:4 * nb],
                                     invnim[:, 6 * nb:8 * nb],
                                     invnim[:, 0:2 * nb])
                # broadcast inv+nim to all partitions: bbq[128, 4nb]
                ps_b = main_ps.tile([128, 4 * nb], F32, tag="ps_s", bufs=1,
                                    name=f"ps_b_{q}")
                nc.tensor.matmul(ps_b[:], ones_row[:], invnim[:, 0:4 * nb],
                                 start=True, stop=True)
                bbq = stats_pool.tile([128, 4 * nb], F32, tag="bbq", bufs=3,
                                      name=f"bbq_{q}")
                nc.vector.tensor_copy(bbq[:], ps_b[:])
                # per-band fp16 bias rows (row 0 of a zeroed [128, .] tile:
                # full-height stationary keeps the preload MM HAM-visible)
                biasqTs, biasaps = {}, {}
                for bi, br in enumerate(("m", "p")):
                    bw = a["bwq"][br]
                    b1c, w1c = cb2[br]
                    biasqT = stats_pool.tile([128, nb * H], FP16, tag="biasqT",
                                             bufs=4, name=f"biasqT_{br}_{q}")
                    if 2 * q + bi < 4:
                        nc.vector.memset(biasqT[:], 0.0)
                    tmpb = stats_pool.tile([1, H], FP16, tag="tmpb", bufs=4,
                                           name=f"tmpb_{br}_{q}")
                    for r in range(nb - 1):
                        # biasT = b1p*std + (-mean)*w1sum
                        nc.vector.tensor_scalar_mul(
                            tmpb[:], bw[:, r * H:(r + 1) * H],
                            invnim[0:1, 4 * nb + bi * nb + r:
                                   4 * nb + bi * nb + r + 1])
                        nc.vector.scalar_tensor_tensor(
                            biasqT[0:1, r * H:(r + 1) * H],
                            bw[:, nb * H + r * H:nb * H + (r + 1) * H],
                            invnim[0:1, 6 * nb + bi * nb + r:
                                   6 * nb + bi * nb + r + 1],
                            tmpb[:],
                            op0=ALU.mult, op1=ALU.add)
                    # last band of the quad-branch runs in AP mode (bias fed
                    # to the tanh as a per-chunk AP; no PSUM preload): these
                    # bands rebalance PE (-4 matmuls) vs ACT (+2 instrs).
                    r = nb - 1
                    k = k0 + r
                    biasap = stats_pool.tile([128, NHC], F32, tag="biasap",
                                             bufs=4, name=f"biasap_{br}_{q}")
                    nc.vector.scalar_tensor_tensor(
                        biasap[:],
                        w1c[:, k * NHC:(k + 1) * NHC],
                        bbq[:, 2 * nb + bi * nb + r:2 * nb + bi * nb + r + 1],
                        b1c[:, k * NHC:(k + 1) * NHC],
                        op0=ALU.mult, op1=ALU.add)
                    biasqTs[br] = biasqT
                    biasaps[br] = biasap
                return dict(**a, bbq=bbq, biasqTs=biasqTs, biasaps=biasaps)

            def bands(st, bi, br):
                """fc1(+bias preload)+tanh+fc2+mask for one quad-branch."""
                q = st["q"]
                k0, nb = QUADS[q]
                w2tp, b2c = cb[br]
                fq, wq = st["fqs"][br], st["wqs"][br]
                bbq, biasqT = st["bbq"], st["biasqTs"][br]
                biasap = st["biasaps"][br]
                fc2g = main_ps.tile([128, T], F32, tag="fc2ps", bufs=1,
                                    name=f"fc2g_{br}_{q}")
                for r in range(nb):
                    k = k0 + r
                    ap_mode = (r == nb - 1)
                    h1sb = h1sb_pool.tile([128, NHC * T], FP16, bufs=6,
                                          tag="h1sb", name=f"h1sb_{br}_{k}")
                    for half in range(2):
                        hps = main_ps.tile([128, 1024], F32, tag="h1ps2",
                                           bufs=3, name=f"h1ps_{br}_{k}_{half}")
                        for hh in range(2):
                            hc = 2 * half + hh
                            if not ap_mode:
                                # bias preload, then fc1 accumulates on top
                                nc.tensor.matmul(
                                    hps[:, hh * T:(hh + 1) * T],
                                    biasqT[:, (r * NHC + hc) * 128:
                                           (r * NHC + hc + 1) * 128],
                                    ones512[:], start=True, stop=False)
                            nc.tensor.matmul(
                                hps[:, hh * T:(hh + 1) * T],
                                wq[:, (r * NHC + hc) * 128:
                                      (r * NHC + hc + 1) * 128],
                                fq[:, r * T:(r + 1) * T],
                                start=ap_mode, stop=True)
                            if ap_mode:
                                nc.scalar.activation(
                                    h1sb[:, hc * T:(hc + 1) * T],
                                    hps[:, hh * T:(hh + 1) * T], AF.Tanh,
                                    bias=biasap[:, hc:hc + 1],
                                    scale=bbq[:, bi * nb + r:bi * nb + r + 1])
                        if not ap_mode:
                            nc.scalar.activation(
                                h1sb[:, half * 1024:(half + 1) * 1024],
                                hps[:], AF.Tanh,
                                scale=bbq[:, bi * nb + r:bi * nb + r + 1])
                    wp, woff = WPADS[k], int(WOFFS[k])
                    for hc in range(NHC):
                        nc.tensor.matmul(
                            fc2g[32 * r:32 * r + wp, :],
                            w2tp[:, hc * WPTOT + woff: hc * WPTOT + woff + wp],
                            h1sb[:, hc * T:(hc + 1) * T],
                            start=(hc == 0), stop=(hc == NHC - 1),
                            tile_position=(0, 32 * r))
                # m branch: sigmoid(x+b2) computed as tanh(0.5x+0.5b2)
                # (b2c_m pre-halved on host); completed in final_chunk.
                # Keeps the ACT table on {tanh, sin} -> zero table swaps.
                grp_t = band_pool.tile([128, T], F32, tag="band",
                                       name=f"grp_{br}_{q}")
                nc.scalar.activation(grp_t[:], fc2g[:], AF.Tanh,
                                     bias=b2c[:, q:q + 1],
                                     scale=0.5 if br == "m" else 1.0)
                for r in range(nb):
                    k = k0 + r
                    w, off = BANDS[k], int(OFFS[k])
                    j0, r0 = off // 128, off % 128
                    if off + w <= (j0 + 1) * 128:
                        nc.sync.dma_start(
                            masks[br][r0:r0 + w, j0 * T:(j0 + 1) * T],
                            grp_t[32 * r:32 * r + w, :])
                    else:
                        n1 = (j0 + 1) * 128 - off
                        nc.sync.dma_start(
                            masks[br][r0:128, j0 * T:(j0 + 1) * T],
                            grp_t[32 * r:32 * r + n1, :])
                        rem = w - n1
                        if j0 + 1 < 2:
                            nc.sync.dma_start(
                                masks[br][0:rem, (j0 + 1) * T:(j0 + 2) * T],
                                grp_t[32 * r + n1:32 * r + w, :])
                        else:
                            nc.sync.dma_start(
                                masks[br + "2"][0:rem, :],
                                grp_t[32 * r + n1:32 * r + w, :])

            st = stats_B(stats_A(0))
            for q in range(len(QUADS)):
                a_next = stats_A(q + 1) if q + 1 < len(QUADS) else None
                bands(st, 0, "m")
                st_next = stats_B(a_next) if a_next is not None else None
                bands(st, 1, "p")
                st = st_next

            # all finals at kernel end: mid-kernel consumption of the
            # DMA-scattered masks races the scatters (the framework's
            # single-sem wait cannot cover many DMA writers across queues).
            final_chunk(0)
            final_chunk(1)
            final_chunk(2)

    nc.compile()
    return nc


def kernel(mag_features, phase_features, noisy_mag, noisy_phase,
           mag_gamma, mag_beta, mag_W1, mag_b1, mag_W2, mag_b2,
           ph_gamma, ph_beta, ph_W1, ph_b1, ph_W2, ph_b2):
    if "nc" not in _cache:
        _cache["nc"] = _build()
    nc = _cache["nc"]

    mW1gT, mbw, mb1c, mw1c, mb2c, mW2Tp = _prep_branch(
        np.asarray(mag_gamma), np.asarray(mag_beta), np.asarray(mag_W1),
        np.asarray(mag_b1), np.asarray(mag_W2), np.asarray(mag_b2))
    mb2c = mb2c * 0.5          # mag sigmoid -> tanh(0.5x + 0.5 b2) trick
    pW1gT, pbw, pb1c, pw1c, pb2c, pW2Tp = _prep_branch(
        np.asarray(ph_gamma), np.asarray(ph_beta), np.asarray(ph_W1),
        np.asarray(ph_b1), np.asarray(ph_W2), np.asarray(ph_b2))

    shared = dict(
        w1gt_m=mW1gT, w2tp_m=mW2Tp, b2c_m=mb2c, bw_m=mbw,
        b1c_m=mb1c, w1c_m=mw1c, b1c_p=pb1c, w1c_p=pw1c,
        w1gt_p=pW1gT, w2tp_p=pW2Tp, b2c_p=pb2c, bw_p=pbw,
        ones_col=np.ones((128, 1), np.float32),
        ones_row=np.ones((1, 128), np.float32),
        ones512=np.ones((128, 512), np.float16),
    )
    mag_features = np.asarray(mag_features)
    phase_features = np.asarray(phase_features)
    noisy_mag = np.asarray(noisy_mag)
    noisy_phase = np.asarray(noisy_phase)

    in_maps = []
    for b in range(B):
        m = dict(shared)
        # [C, T, K] -> [C, K, T] k-major, contiguous per-band slices, fp16
        m["feat_m"] = np.ascontiguousarray(
            mag_features[b].transpose(0, 2, 1)).reshape(C, K * T).astype(
                np.float16)
        m["feat_p"] = np.ascontiguousarray(
            phase_features[b].transpose(0, 2, 1)).reshape(C, K * T).astype(
                np.float16)
        m["noisy_m"] = np.ascontiguousarray(noisy_mag[b]) * np.float32(0.5)
        m["noisy_p"] = np.ascontiguousarray(noisy_phase[b])
        in_maps.append(m)

    import os
    trace = bool(os.environ.get("BASS_PROFILE"))
    res = run_bass_kernel_spmd(nc, in_maps, list(range(B)), trace=trace)
    _cache["last_result"] = res
    out = np.stack([res.results[b]["out"].view(np.complex64) for b in range(B)])
    return out


# revision 61
# speedup vs baseline: 1.0166x; 1.0135x over previous
"""Trainium2 Bass kernel for nn_DualBranchDecoder.

Dual-branch band-split decoder: per-band GroupNorm -> fc1(C=128->H=512)+tanh
-> per-band fc2(H->w_k) -> sigmoid mag mask / tanh phase offset -> complex out.

Sharding: data-parallel over batch B=8 across 8 NeuronCores (one sample per
core).

Design notes (the Activation engine is the structural bottleneck: tanh over
16.25M h1 elements at (N+352)/1.2 ns per instruction; the PE joins it at
~175us once the bias preloads are added):
 - GroupNorm folded into fc1: the matmul consumes RAW fp16 features; the
   per-band 1/std becomes the tanh activation's `scale` operand and the
   exact per-(k,h) bias is injected as biasT = b1p*std - mean*w1sum so that
   tanh(inv*(mm + biasT)) == tanh(inv*mm + bias_true). biasT (an fp16 row,
   built per quad on the DVE) sits in row 0 of a zeroed [128, nb*H]
   stationary; a full-height 128x128 matmul against an all-ones moving
   operand preloads it into PSUM and fc1 accumulates on top (start=False).
   Full-height matters: row-tiled matmuls are invisible to the HAM clock
   gate and freeze the PE at 1.2 GHz.
 - This keeps tanh bias-free, legalizing 1024-wide (2-PSUM-bank) tanh
   instructions. The last band of each quad-branch instead uses a bias-AP +
   512-wide tanh (no preload): 16 such bands rebalance PE vs ACT to ~175us
   each.
 - Stats (bn_stats/bn_aggr + quake-rsqrt chain) run batched per quad for
   both branches and are software-pipelined ONE QUAD AHEAD of the band
   matmuls, so the PE never stalls on the DVE chain (HAM stays warm).
 - mag sigmoid computed as tanh(0.5x+0.5b2) with host-halved noisy_mag
   (sigmoid = 0.5(1+t)), keeping the ACT table on {tanh, sin}: no
   mid-kernel table swaps.
 - cos(x) = sin(x + pi/2) with the +pi/2 folded into the range reduction;
   sin+cos share one bias-free 1024-wide Sin activation per f-chunk.
 - All three final chunks run at the kernel end: the masks tiles have ~30
   DMA writers each and the framework's single-semaphore waits cannot
   express that dependency mid-kernel (measured race).
"""
import sys
sys.path.insert(0, '/opt/trn_rl_repo')

import numpy as np

import concourse.bacc as bacc
import concourse.tile as tile
import concourse.mybir as mybir
from concourse.bass_utils import run_bass_kernel_spmd

F32 = mybir.dt.float32
FP16 = mybir.dt.float16
AF = mybir.ActivationFunctionType
ALU = mybir.AluOpType
I32 = mybir.dt.int32

# problem constants (hardcoded per contract)
B, C, T = 8, 128, 512
BANDS = [2] + [3] * 10 + [8] * 12 + [16] * 7 + [17]
K = len(BANDS)                      # 31
F = sum(BANDS)                      # 257
H = 4 * C                           # 512
NHC = H // 128                      # 4 h-chunks
EPS = 1e-5

OFFS = np.concatenate([[0], np.cumsum(BANDS)]).astype(int)   # band start freqs
WPADS = [w + (w & 1) for w in BANDS]                         # even-M pad
WOFFS = np.concatenate([[0], np.cumsum(WPADS)]).astype(int)
WPTOT = int(WOFFS[-1])

QUADS = [(4 * i, 4) for i in range(7)] + [(28, 3)]
MAGIC = float(1.5 * 2 ** 23)
INV2PI = float(1.0 / (2 * np.pi))
N2PI = float(-2 * np.pi)
PI = float(np.pi)
HALFPI = float(np.pi / 2)

_cache = {}


def _prep_branch(gamma, beta, W1, b1, W2, b2):
    """Host-side constant prep for one branch."""
    # W1gT[c, k*H + h] = W1[k,h,c] * gamma[k,c]
    W1g = W1 * gamma[:, None, :]                      # [K, H, C]
    W1gT = np.ascontiguousarray(W1g.transpose(2, 0, 1).reshape(C, K * H))
    W1gT = W1gT.astype(np.float16)
    # b1p[k,h] = b1[k,h] + sum_c W1[k,h,c]*beta[k,c]
    b1p = b1 + np.einsum('khc,kc->kh', W1, beta)      # [K, H]
    # w1sum[k,h] = sum_c W1g[k,h,c]  (fp16-rounded W1g)
    w1s = W1gT.astype(np.float32).reshape(C, K, H).sum(axis=0)   # [K, H]
    # bw row: per quad, [b1p bands k0..k0+nb | w1sum same], fp16
    bw = np.zeros((1, 2 * K * H), np.float16)
    for q, (k0, nb) in enumerate(QUADS):
        o = 2 * H * k0
        bw[0, o:o + nb * H] = b1p[k0:k0 + nb].reshape(-1).astype(np.float16)
        bw[0, o + nb * H:o + 2 * nb * H] = \
            w1s[k0:k0 + nb].reshape(-1).astype(np.float16)
    b2p = b2.astype(np.float32)
    # broadcast-layout b1p/w1sum for the AP-mode bands: [128, K*NHC] f32
    b1c = np.zeros((128, K * NHC), np.float32)
    w1c = np.zeros((128, K * NHC), np.float32)
    for k in range(K):
        for hc in range(NHC):
            b1c[:, k * NHC + hc] = b1p[k, hc * 128:(hc + 1) * 128]
            w1c[:, k * NHC + hc] = w1s[k, hc * 128:(hc + 1) * 128]
    # W2Tp[p, hc*WPTOT + woff_k + j] = W2[off_k + j, hc*128 + p], zero-pad odd
    W2Tp = np.zeros((128, NHC * WPTOT), np.float32)
    for k in range(K):
        w, off, woff = BANDS[k], OFFS[k], WOFFS[k]
        for hc in range(NHC):
            W2Tp[:, hc * WPTOT + woff: hc * WPTOT + woff + w] = \
                W2[off:off + w, hc * 128:(hc + 1) * 128].T
    W2Tp = W2Tp.astype(np.float16)
    # b2g[32*r + p, q] = b2p[off_{k0+r} + p] (p < w) for quad q
    # (the mag branch later halves this: sigmoid(x+b) == 0.5(1+tanh(.5x+.5b)))
    b2g = np.zeros((128, len(QUADS)), np.float32)
    for q, (k0, nb) in enumerate(QUADS):
        for r in range(nb):
            k = k0 + r
            b2g[32 * r:32 * r + BANDS[k], q] = b2p[OFFS[k]:OFFS[k] + BANDS[k]]
    return W1gT, bw, b1c, w1c, b2g, W2Tp


def _build():
    nc = bacc.Bacc("TRN2", target_bir_lowering=False)

    ins = {}
    for br in ("m", "p"):
        ins[f"feat_{br}"] = nc.dram_tensor(f"feat_{br}", [C, K * T], FP16,
                                           kind="ExternalInput")
        ins[f"w1gt_{br}"] = nc.dram_tensor(f"w1gt_{br}", [C, K * H], FP16,
                                           kind="ExternalInput")
        ins[f"w2tp_{br}"] = nc.dram_tensor(f"w2tp_{br}", [128, NHC * WPTOT],
                                           FP16, kind="ExternalInput")
        # per-quad blocks: [b1p bands k0..k0+nb (nb*H) | w1sum same (nb*H)]
        ins[f"bw_{br}"] = nc.dram_tensor(f"bw_{br}", [1, 2 * K * H], FP16,
                                         kind="ExternalInput")
        ins[f"b2c_{br}"] = nc.dram_tensor(f"b2c_{br}", [128, len(QUADS)], F32,
                                          kind="ExternalInput")
        ins[f"b1c_{br}"] = nc.dram_tensor(f"b1c_{br}", [128, K * NHC], F32,
                                          kind="ExternalInput")
        ins[f"w1c_{br}"] = nc.dram_tensor(f"w1c_{br}", [128, K * NHC], F32,
                                          kind="ExternalInput")
        ins[f"noisy_{br}"] = nc.dram_tensor(f"noisy_{br}", [F, T], F32,
                                            kind="ExternalInput")
    ones_col_d = nc.dram_tensor("ones_col", [128, 1], F32, kind="ExternalInput")
    ones_row_d = nc.dram_tensor("ones_row", [1, 128], F32, kind="ExternalInput")
    ones512_d = nc.dram_tensor("ones512", [128, 512], FP16,
                               kind="ExternalInput")
    out_d = nc.dram_tensor("out", [F, 2 * T], F32, kind="ExternalOutput")

    with tile.TileContext(nc) as tc:
        with (
            tc.tile_pool(name="featk", bufs=3) as featk_pool,
            tc.tile_pool(name="w1t", bufs=3) as w1t_pool,
            tc.tile_pool(name="h1sb", bufs=4) as h1sb_pool,
            tc.tile_pool(name="band", bufs=4) as band_pool,
            tc.tile_pool(name="const", bufs=1) as const_pool,
            tc.tile_pool(name="statsb", bufs=2) as stats_pool,
            tc.tile_pool(name="fin", bufs=1) as fin_pool,
            tc.tile_pool(name="mainps", bufs=1, space="PSUM") as main_ps,
        ):
            # ---- critical path first: quad-0 w1(m) for warmup, then feat ----
            k0_0, nb_0 = QUADS[0]
            wq0 = w1t_pool.tile([128, nb_0 * H], FP16, tag="w1q", name="w1q_m_0")
            nc.sync.dma_start(wq0[:], ins["w1gt_m"][:, k0_0 * H:(k0_0 + nb_0) * H])
            fq0 = featk_pool.tile([128, nb_0 * T], FP16, tag="featq",
                                  name="featq_m_0")
            nc.sync.dma_start(fq0[:], ins["feat_m"][:, k0_0 * T:(k0_0 + nb_0) * T])
            fq0p = featk_pool.tile([128, nb_0 * T], FP16, tag="featq",
                                   name="featq_p_0")
            nc.sync.dma_start(fq0p[:], ins["feat_p"][:, k0_0 * T:(k0_0 + nb_0) * T])

            # ---- constants ----
            ones_col = const_pool.tile([128, 1], F32)
            nc.sync.dma_start(ones_col[:], ones_col_d[:])
            ones_row = const_pool.tile([1, 128], F32)
            nc.sync.dma_start(ones_row[:], ones_row_d[:])
            ones512 = const_pool.tile([128, 512], FP16)
            nc.sync.dma_start(ones512[:], ones512_d[:])

            cb, cb2 = {}, {}
            for br in ("m", "p"):
                w2tp = const_pool.tile([128, NHC * WPTOT], FP16,
                                       tag=f"w2tp_{br}", name=f"w2tp_{br}")
                nc.gpsimd.dma_start(w2tp[:], ins[f"w2tp_{br}"][:])
                b2c = const_pool.tile([128, len(QUADS)], F32, tag=f"b2c_{br}",
                                      name=f"b2c_{br}")
                nc.gpsimd.dma_start(b2c[:], ins[f"b2c_{br}"][:])
                b1c = const_pool.tile([128, K * NHC], F32, tag=f"b1c_{br}",
                                      name=f"b1c_{br}")
                nc.gpsimd.dma_start(b1c[:], ins[f"b1c_{br}"][:])
                w1c = const_pool.tile([128, K * NHC], F32, tag=f"w1c_{br}",
                                      name=f"w1c_{br}")
                nc.gpsimd.dma_start(w1c[:], ins[f"w1c_{br}"][:])
                cb[br] = (w2tp, b2c)
                cb2[br] = (b1c, w1c)

            # ---- PE warm-up: keep the PE busy until the first real fc1
            # matmuls (stats chain latency ~12us) so HAM stays at K=8 ----
            for wi in range(36):
                wps = main_ps.tile([128, 1024], F32, tag="h1ps2", bufs=3,
                                   name=f"warm_{wi}")
                nc.tensor.matmul(wps[:, 0:512], wq0[:, 0:128], wq0[:, 0:512],
                                 start=True, stop=True)

            masks = {}
            for br in ("m", "p"):
                masks[br] = const_pool.tile([128, 2 * T], F32,
                                            tag=f"mask_{br}", name=f"mask_{br}")
                masks[br + "2"] = const_pool.tile([1, T], F32,
                                                  tag=f"mask2_{br}",
                                                  name=f"mask2_{br}")

            # noisy inputs prefetched well before the finals need them
            noisy_t = {}
            for j in range(3):
                rows = 128 if j < 2 else 1
                for nm, src in (("nmag", "noisy_m"), ("nph", "noisy_p")):
                    tl = fin_pool.tile([rows, T], F32, tag=f"{nm}_{j}",
                                       name=f"{nm}_{j}")
                    nc.gpsimd.dma_start(
                        tl[:], ins[src][j * 128:j * 128 + rows, :])
                    noisy_t[(nm, j)] = tl

            def final_chunk(j):
                """Complex assembly for f-chunk j (0,1: 128 rows; 2: 1 row).
                noisy_m arrives pre-halved from the host: sigmoid mask
                = 0.5(1+t) so enh = (t+1)*(0.5*noisy)."""
                ve = nc.vector
                rows = 128 if j < 2 else 1
                if j < 2:
                    mask_ap = masks["m"][:, j * T:(j + 1) * T]
                    poff_ap = masks["p"][:, j * T:(j + 1) * T]
                else:
                    mask_ap = masks["m2"][0:1, :]
                    poff_ap = masks["p2"][0:1, :]
                nmag = noisy_t[("nmag", j)]
                nph = noisy_t[("nph", j)]

                ang = fin_pool.tile([rows, T], F32, tag=f"ang_{j}",
                                    name=f"ang_{j}")
                ve.scalar_tensor_tensor(ang[:], poff_ap, PI, nph[:],
                                        op0=ALU.mult, op1=ALU.add)
                enh = fin_pool.tile([rows, T], F32, tag=f"enh_{j}",
                                    name=f"enh_{j}")
                ve.scalar_tensor_tensor(enh[:], mask_ap, 1.0, nmag[:],
                                        op0=ALU.add, op1=ALU.mult)
                # sincos input tile: cols 0:T = sin arg, T:2T = cos arg
                sc_in = fin_pool.tile([rows, 2 * T], F32, tag=f"sc_in_{j}",
                                      name=f"sc_in_{j}")
                sc_out = fin_pool.tile([rows, 2 * T], F32, tag=f"sc_out_{j}",
                                       name=f"sc_out_{j}")
                # sin: n = round(ang/2pi) via magic; ws = ang - 2pi*n
                t2 = fin_pool.tile([rows, T], F32, tag=f"t2_{j}",
                                   name=f"t2_{j}")
                ve.tensor_scalar(t2[:], ang[:], INV2PI, MAGIC,
                                 op0=ALU.mult, op1=ALU.add)
                m2pin = fin_pool.tile([rows, T], F32, tag=f"m2pin_{j}",
                                      name=f"m2pin_{j}")
                ve.tensor_scalar(m2pin[:], t2[:], MAGIC, N2PI,
                                 op0=ALU.subtract, op1=ALU.mult)
                ve.tensor_add(sc_in[:, 0:T], ang[:], m2pin[:])
                # cos: sin(angc - 2pi*round(angc/2pi)) with angc = ang + pi/2
                angc = fin_pool.tile([rows, T], F32, tag=f"angc_{j}",
                                     name=f"angc_{j}")
                ve.tensor_scalar_add(angc[:], ang[:], HALFPI)
                t2c = fin_pool.tile([rows, T], F32, tag=f"t2_{j}",
                                    name=f"t2c_{j}")
                ve.tensor_scalar(t2c[:], angc[:], INV2PI, MAGIC,
                                 op0=ALU.mult, op1=ALU.add)
                m2pinc = fin_pool.tile([rows, T], F32, tag=f"m2pin_{j}",
                                       name=f"m2pinc_{j}")
                ve.tensor_scalar(m2pinc[:], t2c[:], MAGIC, N2PI,
                                 op0=ALU.subtract, op1=ALU.mult)
                ve.tensor_add(sc_in[:, T:2 * T], angc[:], m2pinc[:])
                nc.scalar.activation(sc_out[:], sc_in[:], AF.Sin)

                ot = fin_pool.tile([rows, 2 * T], F32, tag=f"ot_{j}",
                                   name=f"ot_{j}")
                ot2 = ot[:].rearrange("p (t two) -> p t two", two=2)
                ve.tensor_mul(ot2[:, :, 0], enh[:], sc_out[:, T:2 * T])
                ve.tensor_mul(ot2[:, :, 1], enh[:], sc_out[:, 0:T])
                nc.sync.dma_start(out_d[j * 128:j * 128 + rows, :], ot[:])

            # ---- software-pipelined quads: stats/bias prefetched one
            # quad ahead so the PE never waits on the DVE chain ----
            def stats_A(q):
                """DMAs + per-partition stats + sums for quad q (DVE only)."""
                k0, nb = QUADS[q]
                fqs, wqs, bwq = {}, {}, {}
                for br in ("m", "p"):
                    if q == 0 and br == "m":
                        fqs[br], wqs[br] = fq0, wq0
                    else:
                        if q == 0 and br == "p":
                            fqs[br] = fq0p
                        else:
                            fqs[br] = featk_pool.tile(
                                [128, nb * T], FP16, tag="featq",
                                name=f"featq_{br}_{q}")
                            nc.sync.dma_start(
                                fqs[br][:],
                                ins[f"feat_{br}"][:, k0 * T:(k0 + nb) * T])
                        wqs[br] = w1t_pool.tile([128, nb * H], FP16, tag="w1q",
                                                name=f"w1q_{br}_{q}")
                        nc.sync.dma_start(
                            wqs[br][:],
                            ins[f"w1gt_{br}"][:, k0 * H:(k0 + nb) * H])
                    bwq[br] = stats_pool.tile([1, 2 * nb * H], FP16, tag="bw",
                                              bufs=4, name=f"bw_{br}_{q}")
                    nc.gpsimd.dma_start(
                        bwq[br][:],
                        ins[f"bw_{br}"][:, 2 * H * k0:2 * H * (k0 + nb)])
                sums = stats_pool.tile([128, 4 * nb], F32, tag="sums", bufs=2,
                                       name=f"sums_{q}")
                for bi, br in enumerate(("m", "p")):
                    st_q = stats_pool.tile([128, nb * 6], F32, tag="st_q",
                                           bufs=2, name=f"st_{br}_{q}")
                    ag_q = stats_pool.tile([128, nb * 2], F32, tag="ag_q",
                                           bufs=2, name=f"ag_{br}_{q}")
                    for r in range(nb):
                        nc.vector.bn_stats(st_q[:, r * 6:(r + 1) * 6],
                                           fqs[br][:, r * T:(r + 1) * T])
                        nc.vector.bn_aggr(ag_q[:, r * 2:(r + 1) * 2],
                                          st_q[:, r * 6:(r + 1) * 6])
                    ag3 = ag_q[:].rearrange("c (k two) -> c k two", two=2)
                    mean_ap = ag3[:, :, 0]
                    var_ap = ag3[:, :, 1]
                    o = 2 * nb * bi
                    nc.vector.tensor_copy(sums[:, o:o + nb], mean_ap)
                    tmp = stats_pool.tile([128, nb], F32, tag="tmp", bufs=2,
                                          name=f"tmp_{br}_{q}")
                    nc.vector.tensor_mul(tmp[:], mean_ap, mean_ap)
                    nc.vector.tensor_add(sums[:, o + nb:o + 2 * nb], tmp[:],
                                         var_ap)
                return dict(q=q, fqs=fqs, wqs=wqs, bwq=bwq, sums=sums)

            def stats_B(a):
                """Cross-partition reduce + rsqrt chain + broadcasts + per-band
                bias rows for quad a['q'] (one tiny PE MM pair + DVE)."""
                q = a["q"]
                k0, nb = QUADS[q]
                ps_s = main_ps.tile([1, 4 * nb], F32, tag="ps_s", bufs=1,
                                    name=f"ps_s_{q}")
                nc.tensor.matmul(ps_s[:], ones_col[:], a["sums"][:],
                                 start=True, stop=True)
                g = stats_pool.tile([1, 4 * nb], F32, tag="g", bufs=2,
                                    name=f"g_{q}")
                nc.vector.tensor_scalar_mul(g[:], ps_s[:], 1.0 / C)
                gm2 = stats_pool.tile([1, 2 * nb], F32, tag="gm2", bufs=2,
                                      name=f"gm2_{q}")
                gvar = stats_pool.tile([1, 2 * nb], F32, tag="gvar", bufs=2,
                                       name=f"gvar_{q}")
                gq = g[:].rearrange("c (b two k) -> c b two k", b=2, two=2)
                nc.vector.tensor_mul(gm2[:].rearrange("c (b k) -> c b k", b=2),
                                     gq[:, :, 0, :], gq[:, :, 0, :])
                nc.vector.tensor_sub(gvar[:].rearrange("c (b k) -> c b k", b=2),
                                     gq[:, :, 1, :], gm2[:].rearrange(
                                         "c (b k) -> c b k", b=2))
                # inv = rsqrt(gvar + EPS): quake seed + 3 Newton (pure DVE)
                vv = stats_pool.tile([1, 2 * nb], F32, tag="vv", bufs=2,
                                     name=f"vv_{q}")
                nc.vector.tensor_scalar_add(vv[:], gvar[:], EPS)
                yy = stats_pool.tile([1, 2 * nb], F32, tag="yy", bufs=2,
                                     name=f"yy_{q}")
                nc.vector.tensor_scalar(yy[:].bitcast(I32), vv[:].bitcast(I32),
                                        1, -1, op0=ALU.arith_shift_right,
                                        op1=ALU.bitwise_xor)
                nc.vector.tensor_scalar_add(yy[:].bitcast(I32),
                                            yy[:].bitcast(I32), 0x5f3759e0)
                # invnim[1, 8nb]: [inv | nim=-mean*inv | std | -mean] per
                # branch pair. PSUM preload gets biasT = b1p*std - mean*w1sum
                # so tanh(inv*(mm + biasT)) == tanh(inv*mm + bias_true); the
                # AP-mode bands use bias = b1p + nim*w1sum directly.
                invnim = stats_pool.tile([1, 8 * nb], F32, tag="invnim",
                                         bufs=3, name=f"invnim_{q}")
                tnr = stats_pool.tile([1, 2 * nb], F32, tag="tnr", bufs=2,
                                      name=f"tnr_{q}")
                for it in range(3):
                    nc.vector.tensor_mul(tnr[:], yy[:], yy[:])
                    nc.vector.tensor_mul(tnr[:], tnr[:], vv[:])
                    nc.vector.tensor_scalar(tnr[:], tnr[:], -0.5, 1.5,
                                            op0=ALU.mult, op1=ALU.add)
                    dst = yy[:] if it < 2 else invnim[:, 0:2 * nb]
                    nc.vector.tensor_mul(dst, yy[:], tnr[:])
                # std = (var+eps) * rsqrt(var+eps)
                nc.vector.tensor_mul(invnim[:, 4 * nb:6 * nb], vv[:],
                                     invnim[:, 0:2 * nb])
                nc.vector.tensor_scalar_mul(
                    invnim[:, 6 * nb:8 * nb].rearrange("c (b k) -> c b k", b=2),
                    gq[:, :, 0, :], -1.0)
                nc.vector.tensor_mul(invnim[:, 2 * nb:4 * nb],
                                     invnim[:, 6 * nb:8 * nb],
                                     invnim[:, 0:2 * nb])
                # broadcast inv+nim to all partitions: bbq[128, 4nb]
                ps_b = main_ps.tile([128, 4 * nb], F32, tag="ps_s", bufs=1,
                                    name=f"ps_b_{q}")
                nc.tensor.matmul(ps_b[:], ones_row[:], invnim[:, 0:4 * nb],
                                 start=True, stop=True)
                bbq = stats_pool.tile([128, 4 * nb], F32, tag="bbq", bufs=3,
                                      name=f"bbq_{q}")
                nc.vector.tensor_copy(bbq[:], ps_b[:])
                # per-band fp16 bias rows (row 0 of a zeroed [128, .] tile:
                # full-height stationary keeps the preload MM HAM-visible)
                biasqTs, biasaps = {}, {}
                for bi, br in enumerate(("m", "p")):
                    bw = a["bwq"][br]
                    b1c, w1c = cb2[br]
                    biasqT = stats_pool.tile([128, nb * H], FP16, tag="biasqT",
                                             bufs=4, name=f"biasqT_{br}_{q}")
                    if 2 * q + bi < 4:
                        # zero via the ACT engine: it is idle at startup and
                        # the DVE is on the quad-0 critical path
                        nc.scalar.memzero(biasqT[:])
                    tmpb = stats_pool.tile([1, H], FP16, tag="tmpb", bufs=4,
                                           name=f"tmpb_{br}_{q}")
                    for r in range(nb - 1):
                        # biasT = b1p*std + (-mean)*w1sum
                        nc.vector.tensor_scalar_mul(
                            tmpb[:], bw[:, r * H:(r + 1) * H],
                            invnim[0:1, 4 * nb + bi * nb + r:
                                   4 * nb + bi * nb + r + 1])
                        nc.vector.scalar_tensor_tensor(
                            biasqT[0:1, r * H:(r + 1) * H],
                            bw[:, nb * H + r * H:nb * H + (r + 1) * H],
                            invnim[0:1, 6 * nb + bi * nb + r:
                                   6 * nb + bi * nb + r + 1],
                            tmpb[:],
                            op0=ALU.mult, op1=ALU.add)
                    # last band of the quad-branch runs in AP mode (bias fed
                    # to the tanh as a per-chunk AP; no PSUM preload): these
                    # bands rebalance PE (-4 matmuls) vs ACT (+2 instrs).
                    r = nb - 1
                    k = k0 + r
                    biasap = stats_pool.tile([128, NHC], F32, tag="biasap",
                                             bufs=4, name=f"biasap_{br}_{q}")
                    nc.vector.scalar_tensor_tensor(
                        biasap[:],
                        w1c[:, k * NHC:(k + 1) * NHC],
                        bbq[:, 2 * nb + bi * nb + r:2 * nb + bi * nb + r + 1],
                        b1c[:, k * NHC:(k + 1) * NHC],
                        op0=ALU.mult, op1=ALU.add)
                    biasqTs[br] = biasqT
                    biasaps[br] = biasap
                return dict(**a, bbq=bbq, biasqTs=biasqTs, biasaps=biasaps)

            def bands(st, bi, br):
                """fc1(+bias preload)+tanh+fc2+mask for one quad-branch."""
                q = st["q"]
                k0, nb = QUADS[q]
                w2tp, b2c = cb[br]
                fq, wq = st["fqs"][br], st["wqs"][br]
                bbq, biasqT = st["bbq"], st["biasqTs"][br]
                biasap = st["biasaps"][br]
                fc2g = main_ps.tile([128, T], F32, tag="fc2ps", bufs=1,
                                    name=f"fc2g_{br}_{q}")
                for r in range(nb):
                    k = k0 + r
                    ap_mode = (r == nb - 1)
                    h1sb = h1sb_pool.tile([128, NHC * T], FP16, bufs=6,
                                          tag="h1sb", name=f"h1sb_{br}_{k}")
                    for half in range(2):
                        hps = main_ps.tile([128, 1024], F32, tag="h1ps2",
                                           bufs=3, name=f"h1ps_{br}_{k}_{half}")
                        for hh in range(2):
                            hc = 2 * half + hh
                            if not ap_mode:
                                # bias preload, then fc1 accumulates on top
                                nc.tensor.matmul(
                                    hps[:, hh * T:(hh + 1) * T],
                                    biasqT[:, (r * NHC + hc) * 128:
                                           (r * NHC + hc + 1) * 128],
                                    ones512[:], start=True, stop=False)
                            nc.tensor.matmul(
                                hps[:, hh * T:(hh + 1) * T],
                                wq[:, (r * NHC + hc) * 128:
                                      (r * NHC + hc + 1) * 128],
                                fq[:, r * T:(r + 1) * T],
                                start=ap_mode, stop=True)
                            if ap_mode:
                                nc.scalar.activation(
                                    h1sb[:, hc * T:(hc + 1) * T],
                                    hps[:, hh * T:(hh + 1) * T], AF.Tanh,
                                    bias=biasap[:, hc:hc + 1],
                                    scale=bbq[:, bi * nb + r:bi * nb + r + 1])
                        if not ap_mode:
                            nc.scalar.activation(
                                h1sb[:, half * 1024:(half + 1) * 1024],
                                hps[:], AF.Tanh,
                                scale=bbq[:, bi * nb + r:bi * nb + r + 1])
                    wp, woff = WPADS[k], int(WOFFS[k])
                    for hc in range(NHC):
                        nc.tensor.matmul(
                            fc2g[32 * r:32 * r + wp, :],
                            w2tp[:, hc * WPTOT + woff: hc * WPTOT + woff + wp],
                            h1sb[:, hc * T:(hc + 1) * T],
                            start=(hc == 0), stop=(hc == NHC - 1),
                            tile_position=(0, 32 * r))
                # m branch: sigmoid(x+b2) computed as tanh(0.5x+0.5b2)
                # (b2c_m pre-halved on host); completed in final_chunk.
                # Keeps the ACT table on {tanh, sin} -> zero table swaps.
                grp_t = band_pool.tile([128, T], F32, tag="band",
                                       name=f"grp_{br}_{q}")
                nc.scalar.activation(grp_t[:], fc2g[:], AF.Tanh,
                                     bias=b2c[:, q:q + 1],
                                     scale=0.5 if br == "m" else 1.0)
                for r in range(nb):
                    k = k0 + r
                    w, off = BANDS[k], int(OFFS[k])
                    j0, r0 = off // 128, off % 128
                    if off + w <= (j0 + 1) * 128:
                        nc.sync.dma_start(
                            masks[br][r0:r0 + w, j0 * T:(j0 + 1) * T],
                            grp_t[32 * r:32 * r + w, :])
                    else:
                        n1 = (j0 + 1) * 128 - off
                        nc.sync.dma_start(
                            masks[br][r0:128, j0 * T:(j0 + 1) * T],
                            grp_t[32 * r:32 * r + n1, :])
                        rem = w - n1
                        if j0 + 1 < 2:
                            nc.sync.dma_start(
                                masks[br][0:rem, (j0 + 1) * T:(j0 + 2) * T],
                                grp_t[32 * r + n1:32 * r + w, :])
                        else:
                            nc.sync.dma_start(
                                masks[br + "2"][0:rem, :],
                                grp_t[32 * r + n1:32 * r + w, :])

            st = stats_B(stats_A(0))
            for q in range(len(QUADS)):
                a_next = stats_A(q + 1) if q + 1 < len(QUADS) else None
                bands(st, 0, "m")
                st_next = stats_B(a_next) if a_next is not None else None
                bands(st, 1, "p")
                st = st_next

            # all finals at kernel end: mid-kernel consumption of the
            # DMA-scattered masks races the scatters (the framework's
            # single-sem wait cannot cover many DMA writers across queues).
            final_chunk(0)
            final_chunk(1)
            final_chunk(2)

    nc.compile()
    return nc


def kernel(mag_features, phase_features, noisy_mag, noisy_phase,
           mag_gamma, mag_beta, mag_W1, mag_b1, mag_W2, mag_b2,
           ph_gamma, ph_beta, ph_W1, ph_b1, ph_W2, ph_b2):
    if "nc" not in _cache:
        _cache["nc"] = _build()
    nc = _cache["nc"]

    mW1gT, mbw, mb1c, mw1c, mb2c, mW2Tp = _prep_branch(
        np.asarray(mag_gamma), np.asarray(mag_beta), np.asarray(mag_W1),
        np.asarray(mag_b1), np.asarray(mag_W2), np.asarray(mag_b2))
    mb2c = mb2c * 0.5          # mag sigmoid -> tanh(0.5x + 0.5 b2) trick
    pW1gT, pbw, pb1c, pw1c, pb2c, pW2Tp = _prep_branch(
        np.asarray(ph_gamma), np.asarray(ph_beta), np.asarray(ph_W1),
        np.asarray(ph_b1), np.asarray(ph_W2), np.asarray(ph_b2))

    shared = dict(
        w1gt_m=mW1gT, w2tp_m=mW2Tp, b2c_m=mb2c, bw_m=mbw,
        b1c_m=mb1c, w1c_m=mw1c, b1c_p=pb1c, w1c_p=pw1c,
        w1gt_p=pW1gT, w2tp_p=pW2Tp, b2c_p=pb2c, bw_p=pbw,
        ones_col=np.ones((128, 1), np.float32),
        ones_row=np.ones((1, 128), np.float32),
        ones512=np.ones((128, 512), np.float16),
    )
    mag_features = np.asarray(mag_features)
    phase_features = np.asarray(phase_features)
    noisy_mag = np.asarray(noisy_mag)
    noisy_phase = np.asarray(noisy_phase)

    in_maps = []
    for b in range(B):
        m = dict(shared)
        # [C, T, K] -> [C, K, T] k-major, contiguous per-band slices, fp16
        m["feat_m"] = np.ascontiguousarray(
            mag_features[b].transpose(0, 2, 1)).reshape(C, K * T).astype(
                np.float16)
        m["feat_p"] = np.ascontiguousarray(
            phase_features[b].transpose(0, 2, 1)).reshape(C, K * T).astype(
                np.float16)
        m["noisy_m"] = np.ascontiguousarray(noisy_mag[b]) * np.float32(0.5)
        m["noisy_p"] = np.ascontiguousarray(noisy_phase[b])
        in_maps.append(m)

    import os
    trace = bool(os.environ.get("BASS_PROFILE"))
    res = run_bass_kernel_spmd(nc, in_maps, list(range(B)), trace=trace)
    _cache["last_result"] = res
    out = np.stack([res.results[b]["out"].view(np.complex64) for b in range(B)])
    return out


# revision 62
# speedup vs baseline: 1.0813x; 1.0636x over previous
"""Trainium2 Bass kernel for nn_DualBranchDecoder.

Dual-branch band-split decoder: per-band GroupNorm -> fc1(C=128->H=512)+tanh
-> per-band fc2(H->w_k) -> sigmoid mag mask / tanh phase offset -> complex out.

Sharding: data-parallel over batch B=8 across 8 NeuronCores (one sample per
core). All weight preprocessing (transposes, gamma/beta folding, fp32r
rounding) happens on host; the device does stats, normalize, matmuls (fp32r),
activations and the final complex assembly.
"""
import sys
sys.path.insert(0, '/opt/trn_rl_repo')

import numpy as np
import ml_dtypes

import concourse.bacc as bacc
import concourse.tile as tile
import concourse.mybir as mybir
from concourse.bass_utils import run_bass_kernel_spmd

F32 = mybir.dt.float32
F32R = mybir.dt.float32r
BF16 = mybir.dt.bfloat16
FP16 = mybir.dt.float16
H1DT = FP16
W1DT = FP16
W2DT = FP16
FCDT = FP16
AF = mybir.ActivationFunctionType
ALU = mybir.AluOpType

# problem constants (hardcoded per contract)
B, C, T = 8, 128, 512
BANDS = [2] + [3] * 10 + [8] * 12 + [16] * 7 + [17]
K = len(BANDS)                      # 31
F = sum(BANDS)                      # 257
H = 4 * C                           # 512
NHC = H // 128                      # 4 h-chunks
EPS = 1e-5

OFFS = np.concatenate([[0], np.cumsum(BANDS)]).astype(int)   # band start freqs
WPADS = [w + (w & 1) for w in BANDS]                         # fp32r even-M pad
WOFFS = np.concatenate([[0], np.cumsum(WPADS)]).astype(int)
WPTOT = int(WOFFS[-1])

QUADS = [(4 * i, 4) for i in range(7)] + [(28, 3)]
MAGIC = float(1.5 * 2 ** 23)
INV2PI = float(1.0 / (2 * np.pi))
N2PI = float(-2 * np.pi)
PI = float(np.pi)

_cache = {}


def _round_f32r(x):
    hi = x.astype(ml_dtypes.bfloat16).astype(np.float32)
    lo = (x - hi).astype(ml_dtypes.bfloat16).astype(np.float32)
    return (hi + lo).astype(np.float32)


def _prep_branch(gamma, beta, W1, b1, W2, b2):
    """Host-side constant prep for one branch."""
    # W1gT[c, k*H + h] = W1[k,h,c] * gamma[k,c]
    W1g = W1 * gamma[:, None, :]                      # [K, H, C]
    W1gT = np.ascontiguousarray(W1g.transpose(2, 0, 1).reshape(C, K * H))
    W1gT = W1gT.astype(np.float16)
    # b1p[k,h] = b1[k,h] + sum_c W1[k,h,c]*beta[k,c];  row layout [1, K*H]
    b1p = b1 + np.einsum('khc,kc->kh', W1, beta)      # [K, H]
    b1pT = np.zeros((128, K * NHC), np.float32)
    for k in range(K):
        for hc in range(NHC):
            b1pT[:, k * NHC + hc] = b1p[k, hc * 128:(hc + 1) * 128]
    # W2Tp[p, hc*WPTOT + woff_k + j] = W2[off_k + j, hc*128 + p], zero-pad odd
    W2Tp = np.zeros((128, NHC * WPTOT), np.float32)
    for k in range(K):
        w, off, woff = BANDS[k], OFFS[k], WOFFS[k]
        for hc in range(NHC):
            W2Tp[:, hc * WPTOT + woff: hc * WPTOT + woff + w] = \
                W2[off:off + w, hc * 128:(hc + 1) * 128].T
    W2Tp = W2Tp.astype(np.float16)
    # b2g[32*r + p, q] = b2[off_{k0+r} + p] (p < w) for quad q
    b2g = np.zeros((128, len(QUADS)), np.float32)
    for q, (k0, nb) in enumerate(QUADS):
        for r in range(nb):
            k = k0 + r
            b2g[32 * r:32 * r + BANDS[k], q] = b2[OFFS[k]:OFFS[k] + BANDS[k]]
    return W1gT, b1pT, W2Tp, b2g


def _build():
    nc = bacc.Bacc("TRN2", target_bir_lowering=False)

    # per-core inputs
    ins = {}
    for br in ("m", "p"):
        ins[f"feat_{br}"] = nc.dram_tensor(f"feat_{br}", [C, K * T], F32,
                                           kind="ExternalInput")
        ins[f"w1gt_{br}"] = nc.dram_tensor(f"w1gt_{br}", [C, K * H], W1DT,
                                           kind="ExternalInput")
        ins[f"b1pt_{br}"] = nc.dram_tensor(f"b1pt_{br}", [128, K * NHC], F32,
                                           kind="ExternalInput")
        ins[f"w2tp_{br}"] = nc.dram_tensor(f"w2tp_{br}", [128, NHC * WPTOT], W2DT,
                                           kind="ExternalInput")
        ins[f"b2c_{br}"] = nc.dram_tensor(f"b2c_{br}", [128, len(QUADS)], F32,
                                          kind="ExternalInput")
        ins[f"noisy_{br}"] = nc.dram_tensor(f"noisy_{br}", [F, T], F32,
                                            kind="ExternalInput")
    ones_col_d = nc.dram_tensor("ones_col", [128, 1], F32, kind="ExternalInput")
    ones_row_d = nc.dram_tensor("ones_row", [1, 128], F32, kind="ExternalInput")
    halfpi_d = nc.dram_tensor("halfpi", [128, 1], F32, kind="ExternalInput")
    out_d = nc.dram_tensor("out", [F, 2 * T], F32, kind="ExternalOutput")

    with tile.TileContext(nc) as tc:
        with (
            tc.tile_pool(name="featk", bufs=3) as featk_pool,
            tc.tile_pool(name="w1t", bufs=2) as w1t_pool,
            tc.tile_pool(name="fcent", bufs=4) as fcent_pool,
            tc.tile_pool(name="h1sb", bufs=3) as h1sb_pool,
            tc.tile_pool(name="band", bufs=4) as band_pool,
            tc.tile_pool(name="const", bufs=1) as const_pool,
            tc.tile_pool(name="statsb", bufs=2) as stats_pool,
            tc.tile_pool(name="fin", bufs=2) as fin_pool,
            tc.tile_pool(name="mainps", bufs=1, space="PSUM") as main_ps,
        ):
            # ---- critical-path first: quad-0 mag fetches before anything else ----
            k0_0, nb_0 = QUADS[0]
            fq0 = featk_pool.tile([128, nb_0 * T], F32, tag="featq", name="featq_m_0")
            nc.sync.dma_start(fq0[:], ins["feat_m"][:, k0_0 * T:(k0_0 + nb_0) * T])
            wq0 = w1t_pool.tile([128, nb_0 * H], W1DT, tag="w1q", name="w1q_m_0")
            nc.sync.dma_start(wq0[:], ins["w1gt_m"][:, k0_0 * H:(k0_0 + nb_0) * H])

            # ---- constants ----
            ones_col = const_pool.tile([128, 1], F32)
            nc.sync.dma_start(ones_col[:], ones_col_d[:])
            ones_row = const_pool.tile([1, 128], F32)
            nc.sync.dma_start(ones_row[:], ones_row_d[:])
            halfpi = const_pool.tile([128, 1], F32)
            nc.sync.dma_start(halfpi[:], halfpi_d[:])

            cb = {}
            for br in ("m", "p"):
                b1pt = const_pool.tile([128, K * NHC], F32, tag=f"b1pt_{br}", name=f"b1pt_{br}")
                nc.sync.dma_start(b1pt[:], ins[f"b1pt_{br}"][:])
                w2tp = const_pool.tile([128, NHC * WPTOT], W2DT, tag=f"w2tp_{br}", name=f"w2tp_{br}")
                nc.sync.dma_start(w2tp[:], ins[f"w2tp_{br}"][:])
                b2c = const_pool.tile([128, len(QUADS)], F32, tag=f"b2c_{br}", name=f"b2c_{br}")
                nc.sync.dma_start(b2c[:], ins[f"b2c_{br}"][:])
                cb[br] = (b1pt, w2tp, b2c)

            # ---- PE warm-up: ~5us of continuous matmul to trip HAM un-throttle ----
            for wi in range(16):
                wps = main_ps.tile([128, T], F32, tag="h1ps", bufs=5,
                                   name=f"warm_{wi}")
                nc.tensor.matmul(wps[:], wq0[:, 0:128], wq0[:, 0:T],
                                 start=True, stop=True)

            # ---- fused per-quad stats + band pipeline ----
            masks = {}
            for br in ("m", "p"):
                masks[br] = const_pool.tile([128, 2 * T], F32, tag=f"mask_{br}", name=f"mask_{br}")
                masks[br + "2"] = const_pool.tile([1, T], F32, tag=f"mask2_{br}", name=f"mask2_{br}")

            for q, (k0, nb) in enumerate(QUADS):
                for br in ("m", "p"):
                    b1pt, w2tp, b2c = cb[br]
                    if q == 0 and br == "m":
                        fq, wq = fq0, wq0
                    else:
                        fq = featk_pool.tile([128, nb * T], F32, tag="featq",
                                             name=f"featq_{br}_{q}")
                        nc.sync.dma_start(
                            fq[:], ins[f"feat_{br}"][:, k0 * T:(k0 + nb) * T])
                        wq = w1t_pool.tile([128, nb * H], W1DT, tag="w1q",
                                           name=f"w1q_{br}_{q}")
                        nc.sync.dma_start(
                            wq[:], ins[f"w1gt_{br}"][:, k0 * H:(k0 + nb) * H])

                    # quad stats: per-partition bn stats -> cross-partition sums
                    st_q = stats_pool.tile([128, nb * 6], F32, tag="st_q",
                                           name=f"st_{br}_{q}")
                    ag_q = stats_pool.tile([128, nb * 2], F32, tag="ag_q",
                                           name=f"ag_{br}_{q}")
                    for r in range(nb):
                        nc.vector.bn_stats(st_q[:, r * 6:(r + 1) * 6],
                                           fq[:, r * T:(r + 1) * T])
                        nc.vector.bn_aggr(ag_q[:, r * 2:(r + 1) * 2],
                                          st_q[:, r * 6:(r + 1) * 6])
                    ag3 = ag_q[:].rearrange("c (k two) -> c k two", two=2)
                    mean_ap = ag3[:, :, 0]
                    var_ap = ag3[:, :, 1]
                    sums = stats_pool.tile([128, 2 * nb], F32, tag="sums",
                                           name=f"sums_{br}_{q}")
                    nc.vector.tensor_copy(sums[:, 0:nb], mean_ap)
                    tmp = stats_pool.tile([128, nb], F32, tag="tmp",
                                          name=f"tmp_{br}_{q}")
                    nc.vector.tensor_mul(tmp[:], mean_ap, mean_ap)
                    nc.vector.tensor_add(sums[:, nb:2 * nb], tmp[:], var_ap)
                    ps_s = main_ps.tile([1, 2 * nb], F32, tag="ps_s", bufs=1,
                                        name=f"ps_s_{br}_{q}")
                    nc.tensor.matmul(ps_s[:], ones_col[:], sums[:],
                                     start=True, stop=True)
                    g = stats_pool.tile([1, 2 * nb], F32, tag="g",
                                        name=f"g_{br}_{q}")
                    nc.vector.tensor_scalar_mul(g[:], ps_s[:], 1.0 / C)
                    gm2 = stats_pool.tile([1, nb], F32, tag="gm2",
                                          name=f"gm2_{br}_{q}")
                    nc.vector.tensor_mul(gm2[:], g[:, 0:nb], g[:, 0:nb])
                    gvar = stats_pool.tile([1, nb], F32, tag="gvar",
                                           name=f"gvar_{br}_{q}")
                    nc.vector.tensor_sub(gvar[:], g[:, nb:2 * nb], gm2[:])
                    # inv = rsqrt(gvar + EPS), pure-DVE (quake seed + 3 Newton)
                    vv = stats_pool.tile([1, nb], F32, tag="vv",
                                         name=f"vv_{br}_{q}")
                    nc.vector.tensor_scalar_add(vv[:], gvar[:], EPS)
                    I32 = mybir.dt.int32
                    yy = stats_pool.tile([1, nb], F32, tag="yy",
                                         name=f"yy_{br}_{q}")
                    nc.vector.tensor_scalar(yy[:].bitcast(I32), vv[:].bitcast(I32),
                                            1, -1, op0=ALU.arith_shift_right,
                                            op1=ALU.bitwise_xor)
                    nc.vector.tensor_scalar_add(yy[:].bitcast(I32), yy[:].bitcast(I32),
                                                0x5f3759e0)
                    invmean = stats_pool.tile([1, 2 * nb], F32, tag="invmean",
                                              name=f"invmean_{br}_{q}")
                    tnr = stats_pool.tile([1, nb], F32, tag="tnr",
                                          name=f"tnr_{br}_{q}")
                    for it in range(3):
                        nc.vector.tensor_mul(tnr[:], yy[:], yy[:])
                        nc.vector.tensor_mul(tnr[:], tnr[:], vv[:])
                        nc.vector.tensor_scalar(tnr[:], tnr[:], -0.5, 1.5,
                                                op0=ALU.mult, op1=ALU.add)
                        dst = yy[:] if it < 2 else invmean[:, 0:nb]
                        nc.vector.tensor_mul(dst, yy[:], tnr[:])
                    nc.vector.tensor_copy(invmean[:, nb:2 * nb], g[:, 0:nb])
                    ps_b = main_ps.tile([128, 2 * nb], F32, tag="ps_s", bufs=1,
                                        name=f"ps_b_{br}_{q}")
                    nc.tensor.matmul(ps_b[:], ones_row[:], invmean[:],
                                     start=True, stop=True)
                    bbq = stats_pool.tile([128, 2 * nb], F32, tag="bbq", bufs=3,
                                          name=f"bbq_{br}_{q}")
                    nc.vector.tensor_copy(bbq[:], ps_b[:])
                    # bbq[:, r] = inv ; bbq[:, nb+r] = mean

                    h1s = []
                    for r in range(nb):
                        k = k0 + r
                        fcent = fcent_pool.tile([128, T], FCDT)
                        nc.vector.tensor_scalar(fcent[:], fq[:, r * T:(r + 1) * T],
                                                bbq[:, nb + r:nb + r + 1],
                                                bbq[:, r:r + 1],
                                                op0=ALU.subtract, op1=ALU.mult)
                        h1sb = h1sb_pool.tile([128, NHC * T], H1DT, bufs=6)
                        h1s.append(h1sb)
                        for hc in range(NHC):
                            h1ps = main_ps.tile([128, T], F32, tag="h1ps", bufs=5,
                                                name=f"h1ps_{br}_{k}_{hc}")
                            nc.tensor.matmul(h1ps[:],
                                             wq[:, (r * NHC + hc) * 128:(r * NHC + hc + 1) * 128],
                                             fcent[:], start=True, stop=True)
                            nc.scalar.activation(
                                h1sb[:, hc * T:(hc + 1) * T], h1ps[:],
                                AF.Tanh, bias=b1pt[:, k * NHC + hc:k * NHC + hc + 1])
                    # quad fc2: 4 bands col-tiled into one PSUM bank
                    fc2g = main_ps.tile([128, T], F32, tag="fc2ps", bufs=2,
                                        name=f"fc2g_{br}_{q}")
                    for r in range(nb):
                        k = k0 + r
                        wp, woff = WPADS[k], int(WOFFS[k])
                        for hc in range(NHC):
                            nc.tensor.matmul(
                                fc2g[32 * r:32 * r + wp, :],
                                w2tp[:, hc * WPTOT + woff: hc * WPTOT + woff + wp],
                                h1s[r][:, hc * T:(hc + 1) * T],
                                start=(hc == 0), stop=(hc == NHC - 1),
                                tile_position=(0, 32 * r))
                    grp_t = band_pool.tile([128, T], F32, tag="band")
                    nc.scalar.activation(grp_t[:], fc2g[:],
                                         AF.Sigmoid if br == "m" else AF.Tanh,
                                         bias=b2c[:, q:q + 1])
                    dma_eng = nc.sync if q == len(QUADS) - 1 else nc.gpsimd
                    for r in range(nb):
                        k = k0 + r
                        w, off = BANDS[k], int(OFFS[k])
                        j0, r0 = off // 128, off % 128
                        if off + w <= (j0 + 1) * 128:
                            dma_eng.dma_start(
                                masks[br][r0:r0 + w, j0 * T:(j0 + 1) * T],
                                grp_t[32 * r:32 * r + w, :])
                        else:
                            n1 = (j0 + 1) * 128 - off
                            dma_eng.dma_start(
                                masks[br][r0:128, j0 * T:(j0 + 1) * T],
                                grp_t[32 * r:32 * r + n1, :])
                            rem = w - n1
                            if j0 + 1 < 2:
                                dma_eng.dma_start(
                                    masks[br][0:rem, (j0 + 1) * T:(j0 + 2) * T],
                                    grp_t[32 * r + n1:32 * r + w, :])
                            else:
                                dma_eng.dma_start(masks[br + "2"][0:rem, :],
                                                  grp_t[32 * r + n1:32 * r + w, :])

            # ---- final complex assembly per f-chunk ----
            for j in range(3):
                rows = 128 if j < 2 else 1
                if j < 2:
                    mask_ap = masks["m"][:, j * T:(j + 1) * T]
                    poff_ap = masks["p"][:, j * T:(j + 1) * T]
                else:
                    mask_ap = masks["m2"][0:1, :]
                    poff_ap = masks["p2"][0:1, :]
                nmag = fin_pool.tile([rows, T], F32, tag="nmag")
                nc.gpsimd.dma_start(nmag[:], ins["noisy_m"][j * 128:j * 128 + rows, :])
                nph = fin_pool.tile([rows, T], F32, tag="nph")
                nc.gpsimd.dma_start(nph[:], ins["noisy_p"][j * 128:j * 128 + rows, :])

                ang = fin_pool.tile([rows, T], F32, tag="ang")
                nc.vector.scalar_tensor_tensor(ang[:], poff_ap, PI, nph[:],
                                               op0=ALU.mult, op1=ALU.add)
                enh = fin_pool.tile([rows, T], F32, tag="enh")
                nc.vector.tensor_mul(enh[:], mask_ap, nmag[:])
                # sin: n = round(ang/2pi) via magic; ws = ang - 2pi*n
                t2 = fin_pool.tile([rows, T], F32, tag="t2")
                nc.vector.tensor_scalar(t2[:], ang[:], INV2PI, MAGIC,
                                        op0=ALU.mult, op1=ALU.add)
                m2pin = fin_pool.tile([rows, T], F32, tag="m2pin")
                nc.vector.tensor_scalar(m2pin[:], t2[:], MAGIC, N2PI,
                                        op0=ALU.subtract, op1=ALU.mult)
                nc.vector.tensor_add(m2pin[:], ang[:], m2pin[:])
                sn = fin_pool.tile([rows, T], F32, tag="sn")
                nc.scalar.activation(sn[:], m2pin[:], AF.Sin)
                # cos: n' = round((ang/2pi) + 0.25); wc = ang - 2pi*n'; Sin(wc + pi/2)
                t2c = fin_pool.tile([rows, T], F32, tag="t2c")
                nc.vector.tensor_scalar(t2c[:], ang[:], INV2PI, 0.25,
                                        op0=ALU.mult, op1=ALU.add)
                nc.vector.tensor_scalar_add(t2c[:], t2c[:], MAGIC)
                m2pinc = fin_pool.tile([rows, T], F32, tag="m2pinc")
                nc.vector.tensor_scalar(m2pinc[:], t2c[:], MAGIC, N2PI,
                                        op0=ALU.subtract, op1=ALU.mult)
                nc.vector.tensor_add(m2pinc[:], ang[:], m2pinc[:])
                cn = fin_pool.tile([rows, T], F32, tag="cn")
                nc.scalar.activation(cn[:], m2pinc[:], AF.Sin, bias=halfpi[0:rows, :])

                ot = fin_pool.tile([rows, 2 * T], F32, tag="ot")
                ot2 = ot[:].rearrange("p (t two) -> p t two", two=2)
                nc.vector.tensor_mul(ot2[:, :, 0], enh[:], cn[:])
                nc.vector.tensor_mul(ot2[:, :, 1], enh[:], sn[:])
                nc.sync.dma_start(out_d[j * 128:j * 128 + rows, :], ot[:])

    nc.compile()
    return nc


def kernel(mag_features, phase_features, noisy_mag, noisy_phase,
           mag_gamma, mag_beta, mag_W1, mag_b1, mag_W2, mag_b2,
           ph_gamma, ph_beta, ph_W1, ph_b1, ph_W2, ph_b2):
    if "nc" not in _cache:
        _cache["nc"] = _build()
    nc = _cache["nc"]

    mW1gT, mb1pT, mW2Tp, mb2c = _prep_branch(
        np.asarray(mag_gamma), np.asarray(mag_beta), np.asarray(mag_W1),
        np.asarray(mag_b1), np.asarray(mag_W2), np.asarray(mag_b2))
    pW1gT, pb1pT, pW2Tp, pb2c = _prep_branch(
        np.asarray(ph_gamma), np.asarray(ph_beta), np.asarray(ph_W1),
        np.asarray(ph_b1), np.asarray(ph_W2), np.asarray(ph_b2))

    shared = dict(
        w1gt_m=mW1gT, b1pt_m=mb1pT, w2tp_m=mW2Tp, b2c_m=mb2c,
        w1gt_p=pW1gT, b1pt_p=pb1pT, w2tp_p=pW2Tp, b2c_p=pb2c,
        ones_col=np.ones((128, 1), np.float32),
        ones_row=np.ones((1, 128), np.float32),
        halfpi=np.full((128, 1), np.pi / 2, np.float32),
    )
    mag_features = np.asarray(mag_features)
    phase_features = np.asarray(phase_features)
    noisy_mag = np.asarray(noisy_mag)
    noisy_phase = np.asarray(noisy_phase)

    in_maps = []
    for b in range(B):
        m = dict(shared)
        # [C, T, K] -> [C, K, T] k-major, contiguous per-band slices
        m["feat_m"] = np.ascontiguousarray(
            mag_features[b].transpose(0, 2, 1)).reshape(C, K * T)
        m["feat_p"] = np.ascontiguousarray(
            phase_features[b].transpose(0, 2, 1)).reshape(C, K * T)
        m["noisy_m"] = np.ascontiguousarray(noisy_mag[b])
        m["noisy_p"] = np.ascontiguousarray(noisy_phase[b])
        in_maps.append(m)

    import os
    trace = bool(os.environ.get("BASS_PROFILE"))
    res = run_bass_kernel_spmd(nc, in_maps, list(range(B)), trace=trace)
    _cache["last_result"] = res
    out = np.stack([res.results[b]["out"].view(np.complex64) for b in range(B)])
    return out



# revision 63
# speedup vs baseline: 1.1114x; 1.0279x over previous
"""Trainium2 Bass kernel for nn_DualBranchDecoder.

Dual-branch band-split decoder: per-band GroupNorm -> fc1(C=128->H=512)+tanh
-> per-band fc2(H->w_k) -> sigmoid mag mask / tanh phase offset -> complex out.

Sharding: data-parallel over batch B=8 across 8 NeuronCores (one sample per
core). All weight preprocessing (transposes, gamma/beta folding, fp32r
rounding) happens on host; the device does stats, normalize, matmuls (fp32r),
activations and the final complex assembly.
"""
import sys
sys.path.insert(0, '/opt/trn_rl_repo')

import numpy as np
import ml_dtypes

import concourse.bacc as bacc
import concourse.tile as tile
import concourse.mybir as mybir
from concourse.bass_utils import run_bass_kernel_spmd

F32 = mybir.dt.float32
F32R = mybir.dt.float32r
BF16 = mybir.dt.bfloat16
FP16 = mybir.dt.float16
H1DT = FP16
W1DT = FP16
W2DT = FP16
FCDT = FP16
AF = mybir.ActivationFunctionType
ALU = mybir.AluOpType

# problem constants (hardcoded per contract)
B, C, T = 8, 128, 512
BANDS = [2] + [3] * 10 + [8] * 12 + [16] * 7 + [17]
K = len(BANDS)                      # 31
F = sum(BANDS)                      # 257
H = 4 * C                           # 512
NHC = H // 128                      # 4 h-chunks
EPS = 1e-5

OFFS = np.concatenate([[0], np.cumsum(BANDS)]).astype(int)   # band start freqs
WPADS = [w + (w & 1) for w in BANDS]                         # fp32r even-M pad
WOFFS = np.concatenate([[0], np.cumsum(WPADS)]).astype(int)
WPTOT = int(WOFFS[-1])

QUADS = [(4 * i, 4) for i in range(7)] + [(28, 3)]
MAGIC = float(1.5 * 2 ** 23)
INV2PI = float(1.0 / (2 * np.pi))
N2PI = float(-2 * np.pi)
PI = float(np.pi)
HALFPI = float(np.pi / 2)

_cache = {}


def _round_f32r(x):
    hi = x.astype(ml_dtypes.bfloat16).astype(np.float32)
    lo = (x - hi).astype(ml_dtypes.bfloat16).astype(np.float32)
    return (hi + lo).astype(np.float32)


def _prep_branch(gamma, beta, W1, b1, W2, b2):
    """Host-side constant prep for one branch."""
    # W1gT[c, k*H + h] = W1[k,h,c] * gamma[k,c]
    W1g = W1 * gamma[:, None, :]                      # [K, H, C]
    W1gT = np.ascontiguousarray(W1g.transpose(2, 0, 1).reshape(C, K * H))
    W1gT = W1gT.astype(np.float16)
    # b1p[k,h] = b1[k,h] + sum_c W1[k,h,c]*beta[k,c];  row layout [1, K*H]
    b1p = b1 + np.einsum('khc,kc->kh', W1, beta)      # [K, H]
    b1pT = np.zeros((128, K * NHC), np.float32)
    for k in range(K):
        for hc in range(NHC):
            b1pT[:, k * NHC + hc] = b1p[k, hc * 128:(hc + 1) * 128]
    # W2Tp[p, hc*WPTOT + woff_k + j] = W2[off_k + j, hc*128 + p], zero-pad odd
    W2Tp = np.zeros((128, NHC * WPTOT), np.float32)
    for k in range(K):
        w, off, woff = BANDS[k], OFFS[k], WOFFS[k]
        for hc in range(NHC):
            W2Tp[:, hc * WPTOT + woff: hc * WPTOT + woff + w] = \
                W2[off:off + w, hc * 128:(hc + 1) * 128].T
    W2Tp = W2Tp.astype(np.float16)
    # b2g[32*r + p, q] = b2[off_{k0+r} + p] (p < w) for quad q
    b2g = np.zeros((128, len(QUADS)), np.float32)
    for q, (k0, nb) in enumerate(QUADS):
        for r in range(nb):
            k = k0 + r
            b2g[32 * r:32 * r + BANDS[k], q] = b2[OFFS[k]:OFFS[k] + BANDS[k]]
    return W1gT, b1pT, W2Tp, b2g


def _build():
    nc = bacc.Bacc("TRN2", target_bir_lowering=False)

    # per-core inputs
    ins = {}
    for br in ("m", "p"):
        ins[f"feat_{br}"] = nc.dram_tensor(f"feat_{br}", [C, K * T], F32,
                                           kind="ExternalInput")
        ins[f"w1gt_{br}"] = nc.dram_tensor(f"w1gt_{br}", [C, K * H], W1DT,
                                           kind="ExternalInput")
        ins[f"b1pt_{br}"] = nc.dram_tensor(f"b1pt_{br}", [128, K * NHC], F32,
                                           kind="ExternalInput")
        ins[f"w2tp_{br}"] = nc.dram_tensor(f"w2tp_{br}", [128, NHC * WPTOT], W2DT,
                                           kind="ExternalInput")
        ins[f"b2c_{br}"] = nc.dram_tensor(f"b2c_{br}", [128, len(QUADS)], F32,
                                          kind="ExternalInput")
        ins[f"noisy_{br}"] = nc.dram_tensor(f"noisy_{br}", [F, T], F32,
                                            kind="ExternalInput")
    ones_col_d = nc.dram_tensor("ones_col", [128, 1], F32, kind="ExternalInput")
    ones_row_d = nc.dram_tensor("ones_row", [1, 128], F32, kind="ExternalInput")
    halfpi_d = nc.dram_tensor("halfpi", [128, 1], F32, kind="ExternalInput")
    out_d = nc.dram_tensor("out", [F, 2 * T], F32, kind="ExternalOutput")

    with tile.TileContext(nc) as tc:
        with (
            tc.tile_pool(name="featk", bufs=3) as featk_pool,
            tc.tile_pool(name="w1t", bufs=2) as w1t_pool,
            tc.tile_pool(name="fcent", bufs=4) as fcent_pool,
            tc.tile_pool(name="h1sb", bufs=3) as h1sb_pool,
            tc.tile_pool(name="band", bufs=4) as band_pool,
            tc.tile_pool(name="const", bufs=1) as const_pool,
            tc.tile_pool(name="statsb", bufs=2) as stats_pool,
            tc.tile_pool(name="fin", bufs=2) as fin_pool,
            tc.tile_pool(name="mainps", bufs=1, space="PSUM") as main_ps,
        ):
            # ---- critical-path first: quad-0 mag fetches before anything else ----
            k0_0, nb_0 = QUADS[0]
            fq0 = featk_pool.tile([128, nb_0 * T], F32, tag="featq", name="featq_m_0")
            nc.sync.dma_start(fq0[:], ins["feat_m"][:, k0_0 * T:(k0_0 + nb_0) * T])
            wq0 = w1t_pool.tile([128, nb_0 * H], W1DT, tag="w1q", name="w1q_m_0")
            nc.sync.dma_start(wq0[:], ins["w1gt_m"][:, k0_0 * H:(k0_0 + nb_0) * H])

            # ---- constants ----
            ones_col = const_pool.tile([128, 1], F32)
            nc.sync.dma_start(ones_col[:], ones_col_d[:])
            ones_row = const_pool.tile([1, 128], F32)
            nc.sync.dma_start(ones_row[:], ones_row_d[:])
            halfpi = const_pool.tile([128, 1], F32)
            nc.sync.dma_start(halfpi[:], halfpi_d[:])

            cb = {}
            for br in ("m", "p"):
                b1pt = const_pool.tile([128, K * NHC], F32, tag=f"b1pt_{br}", name=f"b1pt_{br}")
                nc.sync.dma_start(b1pt[:], ins[f"b1pt_{br}"][:])
                w2tp = const_pool.tile([128, NHC * WPTOT], W2DT, tag=f"w2tp_{br}", name=f"w2tp_{br}")
                nc.sync.dma_start(w2tp[:], ins[f"w2tp_{br}"][:])
                b2c = const_pool.tile([128, len(QUADS)], F32, tag=f"b2c_{br}", name=f"b2c_{br}")
                nc.sync.dma_start(b2c[:], ins[f"b2c_{br}"][:])
                cb[br] = (b1pt, w2tp, b2c)

            # ---- PE warm-up: ~5us of continuous matmul to trip HAM un-throttle ----
            for wi in range(16):
                wps = main_ps.tile([128, T], F32, tag="h1ps", bufs=5,
                                   name=f"warm_{wi}")
                nc.tensor.matmul(wps[:], wq0[:, 0:128], wq0[:, 0:T],
                                 start=True, stop=True)

            # ---- fused per-quad stats + band pipeline ----
            masks = {}
            for br in ("m", "p"):
                masks[br] = const_pool.tile([128, 2 * T], F32, tag=f"mask_{br}", name=f"mask_{br}")
                masks[br + "2"] = const_pool.tile([1, T], F32, tag=f"mask2_{br}", name=f"mask2_{br}")

            for q, (k0, nb) in enumerate(QUADS):
                for br in ("m", "p"):
                    b1pt, w2tp, b2c = cb[br]
                    if q == 0 and br == "m":
                        fq, wq = fq0, wq0
                    else:
                        fq = featk_pool.tile([128, nb * T], F32, tag="featq",
                                             name=f"featq_{br}_{q}")
                        nc.sync.dma_start(
                            fq[:], ins[f"feat_{br}"][:, k0 * T:(k0 + nb) * T])
                        wq = w1t_pool.tile([128, nb * H], W1DT, tag="w1q",
                                           name=f"w1q_{br}_{q}")
                        nc.sync.dma_start(
                            wq[:], ins[f"w1gt_{br}"][:, k0 * H:(k0 + nb) * H])

                    # quad stats: per-partition bn stats -> cross-partition sums
                    st_q = stats_pool.tile([128, nb * 6], F32, tag="st_q",
                                           name=f"st_{br}_{q}")
                    ag_q = stats_pool.tile([128, nb * 2], F32, tag="ag_q",
                                           name=f"ag_{br}_{q}")
                    for r in range(nb):
                        nc.vector.bn_stats(st_q[:, r * 6:(r + 1) * 6],
                                           fq[:, r * T:(r + 1) * T])
                        nc.vector.bn_aggr(ag_q[:, r * 2:(r + 1) * 2],
                                          st_q[:, r * 6:(r + 1) * 6])
                    ag3 = ag_q[:].rearrange("c (k two) -> c k two", two=2)
                    mean_ap = ag3[:, :, 0]
                    var_ap = ag3[:, :, 1]
                    sums = stats_pool.tile([128, 2 * nb], F32, tag="sums",
                                           name=f"sums_{br}_{q}")
                    nc.vector.tensor_copy(sums[:, 0:nb], mean_ap)
                    tmp = stats_pool.tile([128, nb], F32, tag="tmp",
                                          name=f"tmp_{br}_{q}")
                    nc.vector.tensor_mul(tmp[:], mean_ap, mean_ap)
                    nc.vector.tensor_add(sums[:, nb:2 * nb], tmp[:], var_ap)
                    ps_s = main_ps.tile([1, 2 * nb], F32, tag="ps_s", bufs=1,
                                        name=f"ps_s_{br}_{q}")
                    nc.tensor.matmul(ps_s[:], ones_col[:], sums[:],
                                     start=True, stop=True)
                    g = stats_pool.tile([1, 2 * nb], F32, tag="g",
                                        name=f"g_{br}_{q}")
                    nc.vector.tensor_scalar_mul(g[:], ps_s[:], 1.0 / C)
                    gm2 = stats_pool.tile([1, nb], F32, tag="gm2",
                                          name=f"gm2_{br}_{q}")
                    nc.vector.tensor_mul(gm2[:], g[:, 0:nb], g[:, 0:nb])
                    gvar = stats_pool.tile([1, nb], F32, tag="gvar",
                                           name=f"gvar_{br}_{q}")
                    nc.vector.tensor_sub(gvar[:], g[:, nb:2 * nb], gm2[:])
                    # inv = rsqrt(gvar + EPS), pure-DVE (quake seed + 3 Newton)
                    vv = stats_pool.tile([1, nb], F32, tag="vv",
                                         name=f"vv_{br}_{q}")
                    nc.vector.tensor_scalar_add(vv[:], gvar[:], EPS)
                    I32 = mybir.dt.int32
                    yy = stats_pool.tile([1, nb], F32, tag="yy",
                                         name=f"yy_{br}_{q}")
                    nc.vector.tensor_scalar(yy[:].bitcast(I32), vv[:].bitcast(I32),
                                            1, -1, op0=ALU.arith_shift_right,
                                            op1=ALU.bitwise_xor)
                    nc.vector.tensor_scalar_add(yy[:].bitcast(I32), yy[:].bitcast(I32),
                                                0x5f3759e0)
                    invmean = stats_pool.tile([1, 2 * nb], F32, tag="invmean",
                                              name=f"invmean_{br}_{q}")
                    tnr = stats_pool.tile([1, nb], F32, tag="tnr",
                                          name=f"tnr_{br}_{q}")
                    for it in range(3):
                        nc.vector.tensor_mul(tnr[:], yy[:], yy[:])
                        nc.vector.tensor_mul(tnr[:], tnr[:], vv[:])
                        nc.vector.tensor_scalar(tnr[:], tnr[:], -0.5, 1.5,
                                                op0=ALU.mult, op1=ALU.add)
                        dst = yy[:] if it < 2 else invmean[:, 0:nb]
                        nc.vector.tensor_mul(dst, yy[:], tnr[:])
                    nc.vector.tensor_copy(invmean[:, nb:2 * nb], g[:, 0:nb])
                    ps_b = main_ps.tile([128, 2 * nb], F32, tag="ps_s", bufs=1,
                                        name=f"ps_b_{br}_{q}")
                    nc.tensor.matmul(ps_b[:], ones_row[:], invmean[:],
                                     start=True, stop=True)
                    bbq = stats_pool.tile([128, 2 * nb], F32, tag="bbq", bufs=3,
                                          name=f"bbq_{br}_{q}")
                    nc.vector.tensor_copy(bbq[:], ps_b[:])
                    # bbq[:, r] = inv ; bbq[:, nb+r] = mean

                    h1s = []
                    for r in range(nb):
                        k = k0 + r
                        fcent = fcent_pool.tile([128, T], FCDT)
                        nc.vector.tensor_scalar(fcent[:], fq[:, r * T:(r + 1) * T],
                                                bbq[:, nb + r:nb + r + 1],
                                                bbq[:, r:r + 1],
                                                op0=ALU.subtract, op1=ALU.mult)
                        h1sb = h1sb_pool.tile([128, NHC * T], H1DT, bufs=6)
                        h1s.append(h1sb)
                        for hc in range(NHC):
                            h1ps = main_ps.tile([128, T], F32, tag="h1ps", bufs=5,
                                                name=f"h1ps_{br}_{k}_{hc}")
                            nc.tensor.matmul(h1ps[:],
                                             wq[:, (r * NHC + hc) * 128:(r * NHC + hc + 1) * 128],
                                             fcent[:], start=True, stop=True)
                            nc.scalar.activation(
                                h1sb[:, hc * T:(hc + 1) * T], h1ps[:],
                                AF.Tanh, bias=b1pt[:, k * NHC + hc:k * NHC + hc + 1])
                    # quad fc2: 4 bands col-tiled into one PSUM bank
                    fc2g = main_ps.tile([128, T], F32, tag="fc2ps", bufs=2,
                                        name=f"fc2g_{br}_{q}")
                    for r in range(nb):
                        k = k0 + r
                        wp, woff = WPADS[k], int(WOFFS[k])
                        for hc in range(NHC):
                            nc.tensor.matmul(
                                fc2g[32 * r:32 * r + wp, :],
                                w2tp[:, hc * WPTOT + woff: hc * WPTOT + woff + wp],
                                h1s[r][:, hc * T:(hc + 1) * T],
                                start=(hc == 0), stop=(hc == NHC - 1),
                                tile_position=(0, 32 * r))
                    # mag: sigmoid(x+b2) = 0.5(1+tanh(0.5x+0.5b2));
                    # b2c_m pre-halved on host, completed in the final via
                    # host-halved noisy_mag. Keeps the ACT table on
                    # {tanh, sin}: no mid-kernel table swaps.
                    grp_t = band_pool.tile([128, T], F32, tag="band")
                    nc.scalar.activation(grp_t[:], fc2g[:], AF.Tanh,
                                         bias=b2c[:, q:q + 1],
                                         scale=0.5 if br == "m" else 1.0)
                    dma_eng = nc.sync if q == len(QUADS) - 1 else nc.gpsimd
                    for r in range(nb):
                        k = k0 + r
                        w, off = BANDS[k], int(OFFS[k])
                        j0, r0 = off // 128, off % 128
                        if off + w <= (j0 + 1) * 128:
                            dma_eng.dma_start(
                                masks[br][r0:r0 + w, j0 * T:(j0 + 1) * T],
                                grp_t[32 * r:32 * r + w, :])
                        else:
                            n1 = (j0 + 1) * 128 - off
                            dma_eng.dma_start(
                                masks[br][r0:128, j0 * T:(j0 + 1) * T],
                                grp_t[32 * r:32 * r + n1, :])
                            rem = w - n1
                            if j0 + 1 < 2:
                                dma_eng.dma_start(
                                    masks[br][0:rem, (j0 + 1) * T:(j0 + 2) * T],
                                    grp_t[32 * r + n1:32 * r + w, :])
                            else:
                                dma_eng.dma_start(masks[br + "2"][0:rem, :],
                                                  grp_t[32 * r + n1:32 * r + w, :])

            # ---- final complex assembly per f-chunk ----
            for j in range(3):
                rows = 128 if j < 2 else 1
                if j < 2:
                    mask_ap = masks["m"][:, j * T:(j + 1) * T]
                    poff_ap = masks["p"][:, j * T:(j + 1) * T]
                else:
                    mask_ap = masks["m2"][0:1, :]
                    poff_ap = masks["p2"][0:1, :]
                nmag = fin_pool.tile([rows, T], F32, tag="nmag")
                nc.gpsimd.dma_start(nmag[:], ins["noisy_m"][j * 128:j * 128 + rows, :])
                nph = fin_pool.tile([rows, T], F32, tag="nph")
                nc.gpsimd.dma_start(nph[:], ins["noisy_p"][j * 128:j * 128 + rows, :])

                ang = fin_pool.tile([rows, T], F32, tag="ang")
                nc.vector.scalar_tensor_tensor(ang[:], poff_ap, PI, nph[:],
                                               op0=ALU.mult, op1=ALU.add)
                # mag mask stored as t=tanh(.5x+.5b2); noisy_m pre-halved:
                # enh = (t+1) * (0.5*noisy)
                enh = fin_pool.tile([rows, T], F32, tag="enh")
                nc.vector.scalar_tensor_tensor(enh[:], mask_ap, 1.0, nmag[:],
                                               op0=ALU.add, op1=ALU.mult)
                # sincos tile: cols 0:T = sin arg, T:2T = cos arg; one act
                sc_in = fin_pool.tile([rows, 2 * T], F32, tag="sc_in")
                sc_out = fin_pool.tile([rows, 2 * T], F32, tag="sc_out")
                # sin: n = round(ang/2pi) via magic; ws = ang - 2pi*n
                t2 = fin_pool.tile([rows, T], F32, tag="t2")
                nc.vector.tensor_scalar(t2[:], ang[:], INV2PI, MAGIC,
                                        op0=ALU.mult, op1=ALU.add)
                m2pin = fin_pool.tile([rows, T], F32, tag="m2pin")
                nc.vector.tensor_scalar(m2pin[:], t2[:], MAGIC, N2PI,
                                        op0=ALU.subtract, op1=ALU.mult)
                nc.vector.tensor_add(sc_in[:, 0:T], ang[:], m2pin[:])
                # cos: sin(angc - 2pi*round(angc/2pi)), angc = ang + pi/2
                angc = fin_pool.tile([rows, T], F32, tag="angc")
                nc.vector.tensor_scalar_add(angc[:], ang[:], HALFPI)
                t2c = fin_pool.tile([rows, T], F32, tag="t2c")
                nc.vector.tensor_scalar(t2c[:], angc[:], INV2PI, MAGIC,
                                        op0=ALU.mult, op1=ALU.add)
                m2pinc = fin_pool.tile([rows, T], F32, tag="m2pinc")
                nc.vector.tensor_scalar(m2pinc[:], t2c[:], MAGIC, N2PI,
                                        op0=ALU.subtract, op1=ALU.mult)
                nc.vector.tensor_add(sc_in[:, T:2 * T], angc[:], m2pinc[:])
                nc.scalar.activation(sc_out[:], sc_in[:], AF.Sin)

                ot = fin_pool.tile([rows, 2 * T], F32, tag="ot")
                ot2 = ot[:].rearrange("p (t two) -> p t two", two=2)
                nc.vector.tensor_mul(ot2[:, :, 0], enh[:], sc_out[:, T:2 * T])
                nc.vector.tensor_mul(ot2[:, :, 1], enh[:], sc_out[:, 0:T])
                nc.sync.dma_start(out_d[j * 128:j * 128 + rows, :], ot[:])

    nc.compile()
    return nc


def kernel(mag_features, phase_features, noisy_mag, noisy_phase,
           mag_gamma, mag_beta, mag_W1, mag_b1, mag_W2, mag_b2,
           ph_gamma, ph_beta, ph_W1, ph_b1, ph_W2, ph_b2):
    if "nc" not in _cache:
        _cache["nc"] = _build()
    nc = _cache["nc"]

    mW1gT, mb1pT, mW2Tp, mb2c = _prep_branch(
        np.asarray(mag_gamma), np.asarray(mag_beta), np.asarray(mag_W1),
        np.asarray(mag_b1), np.asarray(mag_W2), np.asarray(mag_b2))
    mb2c = mb2c * 0.5      # sigmoid -> tanh(0.5x + 0.5 b2) trick
    pW1gT, pb1pT, pW2Tp, pb2c = _prep_branch(
        np.asarray(ph_gamma), np.asarray(ph_beta), np.asarray(ph_W1),
        np.asarray(ph_b1), np.asarray(ph_W2), np.asarray(ph_b2))

    shared = dict(
        w1gt_m=mW1gT, b1pt_m=mb1pT, w2tp_m=mW2Tp, b2c_m=mb2c,
        w1gt_p=pW1gT, b1pt_p=pb1pT, w2tp_p=pW2Tp, b2c_p=pb2c,
        ones_col=np.ones((128, 1), np.float32),
        ones_row=np.ones((1, 128), np.float32),
        halfpi=np.full((128, 1), np.pi / 2, np.float32),
    )
    mag_features = np.asarray(mag_features)
    phase_features = np.asarray(phase_features)
    noisy_mag = np.asarray(noisy_mag)
    noisy_phase = np.asarray(noisy_phase)

    in_maps = []
    for b in range(B):
        m = dict(shared)
        # [C, T, K] -> [C, K, T] k-major, contiguous per-band slices
        m["feat_m"] = np.ascontiguousarray(
            mag_features[b].transpose(0, 2, 1)).reshape(C, K * T)
        m["feat_p"] = np.ascontiguousarray(
            phase_features[b].transpose(0, 2, 1)).reshape(C, K * T)
        m["noisy_m"] = np.ascontiguousarray(noisy_mag[b]) * np.float32(0.5)
        m["noisy_p"] = np.ascontiguousarray(noisy_phase[b])
        in_maps.append(m)

    import os
    trace = bool(os.environ.get("BASS_PROFILE"))
    res = run_bass_kernel_spmd(nc, in_maps, list(range(B)), trace=trace)
    _cache["last_result"] = res
    out = np.stack([res.results[b]["out"].view(np.complex64) for b in range(B)])
    return out

